# revision 27
# baseline (speedup 1.0000x reference)
"""Trainium2 Bass kernel for causal multi-head attention with ALiBi.

Computes, for x:[B,S,D]:
    qkv = x @ W_packed.T + b_packed ; q,k,v = split(qkv)
    heads -> scores = q k^T / sqrt(hd) + alibi_causal_bias
    out = softmax(scores) v -> merge heads -> out @ W_out.T + b_out

Sharding (8 cores): core c handles batch c//4 and heads {k, k+4, k+8, k+12}
(k = c%4), one head per "slot". Slot block-schedules are head-independent
(sized for the largest ALiBi window in the slot), so one SPMD program runs
on all 8 cores; only the data (weight slices, bias tables) differs.
Host sums the 4 out-projection partials per batch and adds
b_out + W_out @ b_v (the v-bias term commutes through attention).

ALiBi sparsity: head h attends effectively only a bounded window back;
dropped softmax mass is <= e^-8 at the worst (group-start) columns.
Slots keep only the causal k-blocks within that window (KEEP blocks).

Softmax without row-max: scores are O(+-6), and exp is recentred per
q-group by C_g (a per-column constant that cancels in normalization),
keeping exp args in fp32 range. In the transposed layout scoresT[k,q] the
recentred ALiBi bias slope*(k - C_g) is per-partition, so it rides the
single Exp activation for free. Row sums come from a ones-row appended to
v in the attn@v matmul; normalization divides by that row.
"""

import os
import sys

import numpy as np

for _p in ("/opt/trn_rl_repo",):
    if os.path.isdir(_p) and _p not in sys.path:
        sys.path.append(_p)

import concourse.bacc as bacc
import concourse.bass as bass
import concourse.tile as tile
from concourse import mybir
from concourse.bass_utils import run_bass_kernel_spmd

B, S, D, H, HD = 2, 2048, 1024, 16, 64
NBLK = S // 128          # 16 k/q blocks
NCORES = 8

F32 = mybir.dt.float32
F32R = mybir.dt.float32r
BF16 = mybir.dt.bfloat16

# Slots A..D: per-core heads [12+k, 8+k, 4+k, k].  KEEP = causal k-blocks
# kept per q-block (max over the slot's heads).  W = q-group width.
SLOT_KEEP = (17, 8, 5, 2)
SLOT_W = (512, 512, 512, 128)
SLOT_OFF0 = (128, 128, 128, 64)
SLOT_TABW = tuple(k + 3 if w == 512 else k for k, w in zip(SLOT_KEEP, SLOT_W))
SLOT_TABOFF = tuple(int(np.cumsum((0,) + SLOT_TABW)[i]) for i in range(4))
TABW = int(sum(SLOT_TABW))


def _slot_schedule(s):
    """Yield (g, q0, W, [(j, lo, hi, tabcol, isdiag), ...]) per q-group."""
    K, W, _ = SLOT_KEEP[s], SLOT_W[s], SLOT_OFF0[s]
    out = []
    if W == 512:
        for g in range(S // 512):
            jlo = max(0, 4 * g + 3 - (K - 1))
            blocks = []
            for j in range(jlo, 4 * g + 4):
                lo = max(0, (j - 4 * g) * 128)
                hi = min(512, (j - 4 * g + K) * 128)
                m = j - 4 * g + (K - 1)
                blocks.append((j, lo, hi, SLOT_TABOFF[s] + m, j >= 4 * g))
            out.append((g, g * 512, 512, blocks))
    else:
        for i in range(NBLK):
            blocks = []
            for j in range(max(0, i - (K - 1)), i + 1):
                m = j - i + (K - 1)
                blocks.append((j, 0, 128, SLOT_TABOFF[s] + m, j == i))
            out.append((i, i * 128, 128, blocks))
    return out


def build_program():
    nc = bacc.Bacc("TRN2", target_bir_lowering=False, debug=False,
                   num_devices=NCORES)

    xT = nc.dram_tensor("xT", [D, S], BF16, kind="ExternalInput")
    wqkvT = nc.dram_tensor("wqkvT", [D, 768], BF16, kind="ExternalInput")
    woT = nc.dram_tensor("woT", [128, 2 * D], BF16, kind="ExternalInput")
    btabq = nc.dram_tensor("btabq", [128, TABW + 4], F32, kind="ExternalInput")
    out = nc.dram_tensor("out", [S, D], BF16, kind="ExternalOutput")

    with tile.TileContext(nc) as tc:
        with tc.tile_pool(name="persist", bufs=1) as pp:
            qkT = [pp.tile([128, S], BF16, tag=f"qkT{t}", name=f"qkT{t}")
                   for t in range(4)]
            v_t = pp.tile([128, 4, NBLK, 65], BF16, tag="v", name="v")
            hoT = [pp.tile([128, S], BF16, tag=f"hoT{t}", name=f"hoT{t}")
                   for t in range(2)]
            btab_sb = pp.tile([128, TABW + 4], F32, tag="btab", name="btab")
            ones_r = pp.tile([65, 64], F32, tag="ones_r", name="ones_r")
            wo_sb = pp.tile([128, 2 * D], BF16, tag="wo", name="wo")

            nc.gpsimd.memset(v_t[:, :, :, 64:65], 1.0)
            nc.gpsimd.memset(ones_r[:], 1.0)
            # touch the ACT engine once while it is idle so the activation
            # table load happens here, not on the first real Exp/Identity
            warm = pp.tile([1, 1], F32, tag="warm", name="warm")
            nc.scalar.activation(warm[:], ones_r[0:1, 0:1],
                                 mybir.ActivationFunctionType.Exp)

            # PSUM: 8 banks as 4 tags; phase-1 QKV borrows all four tags
            with (
                tc.tile_pool(name="xw", bufs=1) as xw,
                tc.tile_pool(name="et", bufs=8) as etp,
                tc.tile_pool(name="nrm", bufs=3) as nrm,
                tc.tile_pool(name="ob", bufs=2) as obp,
                tc.tile_pool(name="ps_sc", bufs=3, space="PSUM") as sc_ps,
                tc.tile_pool(name="ps_av", bufs=2, space="PSUM") as av_ps,
                tc.tile_pool(name="ps_bp", bufs=1, space="PSUM") as bp_ps,
                tc.tile_pool(name="ps_op", bufs=2, space="PSUM") as op_ps,
            ):
                # input stream: (wqkv_m, x_m) pairs first -- qkv phase 1
                # consumes chunk-major right behind the stream -- then the
                # small tables and the out-proj weight (needed much later).
                xT_sb, wqkv_sb = [], []
                for m in range(8):
                    t = xw.tile([128, 768], BF16, tag=f"wqkv{m}",
                                name=f"wqkv{m}")
                    nc.sync.dma_start(t[:], wqkvT[m * 128:(m + 1) * 128, :])
                    wqkv_sb.append(t)
                    t = xw.tile([128, S], BF16, tag=f"x{m}", name=f"x{m}")
                    nc.sync.dma_start(t[:], xT[m * 128:(m + 1) * 128, :])
                    xT_sb.append(t)
                nc.sync.dma_start(btab_sb[:], btabq[:])
                nc.sync.dma_start(wo_sb[:], woT[:])

                def p1_tile(i, w):
                    pool, tag = [(sc_ps, "sc"), (sc_ps, "sc"), (sc_ps, "sc"),
                                 (bp_ps, "bps"), (av_ps, "av"), (av_ps, "av"),
                                 (op_ps, "op"), (op_ps, "op")][i]
                    return pool.tile([128, w], F32, tag=tag, name=f"p1_{i}")

                def qk_half(half):
                    # quarters {2h,2h+1} x 4 f-tiles -> 8 one-bank psums
                    # (m-outer: first matmul waits only for chunk-0 DMAs)
                    pss = {}
                    for ft in range(4):
                        for qi in range(2):
                            pss[ft, qi] = p1_tile(ft * 2 + qi, 512)
                    for m in range(8):
                        for ft in range(4):
                            for qi in range(2):
                                q4 = half * 2 + qi
                                nc.tensor.matmul(
                                    pss[ft, qi][:],
                                    wqkv_sb[m][:, ft * 128:(ft + 1) * 128],
                                    xT_sb[m][:, q4 * 512:(q4 + 1) * 512],
                                    start=(m == 0), stop=(m == 7),
                                )
                    # drain ft=3 first: its psums sit on the "op" tag the
                    # v/qk1 filler units need next.  ft=3 and qi=1 drains
                    # ride ACT (idle here) to shorten the DVE chain.
                    for ft in (3, 0, 1, 2):
                        for qi in range(2):
                            q4 = half * 2 + qi
                            scol = slice(q4 * 512, (q4 + 1) * 512)
                            # psum*scale + bias (1/sqrt(hd) folded into q)
                            if qi == 1 or ft == 3:
                                nc.scalar.activation(
                                    qkT[ft][:, scol], pss[ft, qi][:],
                                    mybir.ActivationFunctionType.Identity,
                                    bias=btab_sb[:, TABW + ft:TABW + ft + 1],
                                    scale=(0.125 if ft < 2 else 1.0),
                                )
                            else:
                                nc.vector.tensor_scalar(
                                    out=qkT[ft][:, scol], in0=pss[ft, qi][:],
                                    scalar1=(0.125 if ft < 2 else 1.0),
                                    scalar2=btab_sb[:, TABW + ft:
                                                    TABW + ft + 1],
                                    op0=mybir.AluOpType.mult,
                                    op1=mybir.AluOpType.add,
                                )

                def v_unit(sb):
                    # one k-block of v for all 4 slots; m-inner, 1/2 bank
                    ps = op_ps.tile([128, 256], F32, tag="op", name="vps")
                    for m in range(8):
                        nc.tensor.matmul(
                            ps[:],
                            xT_sb[m][:, sb * 128:(sb + 1) * 128],
                            wqkv_sb[m][:, 512:768],
                            start=(m == 0), stop=(m == 7),
                        )
                    nc.vector.tensor_copy(
                        v_t[:, :, sb, 0:64],
                        ps[:].rearrange("p (s c) -> p s c", s=4),
                    )

                def qk1_unit(ft, qi):
                    # one (f-tile, q-quarter) of the second qk half; 1 bank
                    q4 = 2 + qi
                    ps = op_ps.tile([128, 512], F32, tag="op", name="qk1ps")
                    for m in range(8):
                        nc.tensor.matmul(
                            ps[:],
                            wqkv_sb[m][:, ft * 128:(ft + 1) * 128],
                            xT_sb[m][:, q4 * 512:(q4 + 1) * 512],
                            start=(m == 0), stop=(m == 7),
                        )
                    scol = slice(q4 * 512, (q4 + 1) * 512)
                    if ft >= 2:
                        nc.scalar.activation(
                            qkT[ft][:, scol], ps[:],
                            mybir.ActivationFunctionType.Identity,
                            bias=btab_sb[:, TABW + ft:TABW + ft + 1],
                            scale=1.0,
                        )
                    else:
                        nc.vector.tensor_scalar(
                            out=qkT[ft][:, scol], in0=ps[:],
                            scalar1=0.125,
                            scalar2=btab_sb[:, TABW + ft:TABW + ft + 1],
                            op0=mybir.AluOpType.mult,
                            op1=mybir.AluOpType.add,
                        )

                qk_half(0)
                for sb in range(4):
                    v_unit(sb)

                # filler units: PE work spliced into the exp-paced attention
                # stream.  Order respects data deps (g2 needs qk1 qi=0 and
                # v 8-11, g3 needs qi=1 and v 12-15; all are emitted at
                # least one full q-group before first use).
                from collections import deque
                fillers = deque()
                for sb in range(4, 8):
                    fillers.append(("v", sb))
                for ft in range(4):
                    fillers.append(("qk1", ft, 0))
                for sb in range(8, 12):
                    fillers.append(("v", sb))
                for ft in range(4):
                    fillers.append(("qk1", ft, 1))
                for sb in range(12, 16):
                    fillers.append(("v", sb))

                def emit_filler():
                    if not fillers:
                        return
                    u = fillers.popleft()
                    if u[0] == "v":
                        v_unit(u[1])
                    elif u[0] == "qk1":
                        qk1_unit(u[1], u[2])
                    else:
                        op_block(u[1])

                # ---- attention (+ out-proj interleaved per q-group) ----
                sched = [_slot_schedule(s) for s in range(4)]

                def scores_av(s, ent, av, coff):
                    """Scores+exp+AV for one q-group into av[:, coff:+W].

                    Depth-1 software pipeline: block bi+1's score matmul is
                    emitted before block bi's AV matmul, so the (in-order)
                    PE isn't stalled on the exp the AV depends on."""
                    po = (s % 2) * 64
                    qT_s = qkT[s // 2][po:po + 64, :]
                    kT_s = qkT[2 + s // 2][po:po + 64, :]
                    g, q0, W, blocks = ent
                    pend = None
                    for bi, (j, lo, hi, tcol, isdiag) in enumerate(blocks):
                        # partial widths: block 0 always spans [0:W] (sets
                        # has_written on the full av range); later blocks
                        # touch only their causal window [lo:hi].
                        sc = sc_ps.tile([128, W], F32, tag="sc", name="sc")
                        nc.tensor.matmul(
                            sc[:, lo:hi],
                            kT_s[:, j * 128:(j + 1) * 128],
                            qT_s[:, q0 + lo:q0 + hi],
                        )
                        et = etp.tile([128, W], BF16, tag="et", name="et")
                        nc.scalar.activation(
                            et[:, lo:hi], sc[:, lo:hi],
                            mybir.ActivationFunctionType.Exp,
                            bias=btab_sb[:, tcol:tcol + 1], scale=1.0,
                        )
                        if isdiag:
                            # zero k>q inside the diagonal 128x128 block
                            nc.gpsimd.affine_select(
                                out=et[:, lo:lo + 128],
                                in_=et[:, lo:lo + 128],
                                compare_op=mybir.AluOpType.is_ge,
                                fill=0.0, base=0,
                                pattern=[[1, 128]],
                                channel_multiplier=-1,
                            )
                        if pend is not None:
                            pbi, pj, plo, phi, pet = pend
                            nc.tensor.matmul(
                                av[:, coff + plo:coff + phi],
                                v_t[:, s, pj, :], pet[:, plo:phi],
                                start=(pbi == 0), stop=False,
                            )
                        pend = (bi, j, lo, hi, et)
                    pbi, pj, plo, phi, pet = pend
                    nc.tensor.matmul(
                        av[:, coff + plo:coff + phi], v_t[:, s, pj, :],
                        pet[:, plo:phi], start=(pbi == 0), stop=True,
                    )

                def norm(s, av, q0, W):
                    """Divide av[0:64] by the ones-row sum; write hoT."""
                    po = (s % 2) * 64
                    hoT_s = hoT[s // 2]
                    lr = nrm.tile([65, W], F32R, tag="lr", name="lr")
                    nc.vector.tensor_copy(lr[64:65, :], av[64:65, :])
                    bps = bp_ps.tile([64, W], F32, tag="bps", name="bps")
                    nc.tensor.matmul(
                        bps[:], ones_r[64:65, 0:64].bitcast(F32R),
                        lr[64:65, :])
                    binv = nrm.tile([64, W], F32, tag="binv", name="binv")
                    nc.vector.reciprocal_approx_fast(out=binv[:], in_=bps[:])
                    if po == 0:
                        nc.vector.tensor_mul(
                            hoT_s[0:64, q0:q0 + W], av[0:64, :], binv[:])
                    else:
                        # DVE lanes can't shift partitions; bounce via DMA
                        tmp = nrm.tile([64, W], BF16, tag="hotmp",
                                       name="hotmp")
                        nc.vector.tensor_mul(tmp[:], av[0:64, :], binv[:])
                        nc.gpsimd.dma_start(
                            hoT_s[64:128, q0:q0 + W], tmp[:])

                def attn_group(s, ent):
                    g, q0, W, blocks = ent
                    av = av_ps.tile([65, W], F32, tag="av", name="av")
                    scores_av(s, ent, av, 0)
                    norm(s, av, q0, W)

                def op_block(sb):
                    ob = obp.tile([128, D], BF16, tag="ob", name="ob")
                    # cc-outer: consecutive matmuls share the stationary
                    # hoT slice (halves the LDWEIGHTS traffic on HW).
                    # Tail blocks borrow the freed sc pool for pipelining.
                    pool = sc_ps if sb >= 12 else op_ps
                    tag = "sc" if pool is sc_ps else "op"
                    pss = [pool.tile([128, 512], F32, tag=tag, name="op")
                           for _ in range(2)]
                    for cc in range(2):
                        for jh in range(2):
                            nc.tensor.matmul(
                                pss[jh][:],
                                hoT[cc][:, sb * 128:(sb + 1) * 128],
                                wo_sb[:, cc * D + jh * 512:
                                      cc * D + (jh + 1) * 512],
                                start=(cc == 0), stop=(cc == 1),
                            )
                    for jh in range(2):
                        if jh == 0 and sb >= 12:
                            nc.scalar.copy(ob[:, 0:512], pss[0][:])
                        else:
                            nc.vector.tensor_copy(
                                ob[:, jh * 512:(jh + 1) * 512], pss[jh][:])
                    nc.gpsimd.dma_start(out[sb * 128:(sb + 1) * 128, :],
                                        ob[:])

                for g in range(4):
                    nfill = 1 if g < 2 else 2
                    # slot D first: its norm bounces hoT rows through a
                    # DMA whose latency then hides under slots A-C
                    avD = av_ps.tile([65, 512], F32, tag="av", name="avD")
                    for i4 in range(4):
                        scores_av(3, sched[3][4 * g + i4], avD, i4 * 128)
                        for _ in range(nfill):
                            emit_filler()
                    norm(3, avD, g * 512, 512)
                    for s in range(3):
                        attn_group(s, sched[s][g])
                        for _ in range(nfill):
                            emit_filler()
                    for sb in range(4 * g, 4 * g + 4):
                        fillers.append(("op", sb))
                while fillers:
                    emit_filler()

    nc.compile()
    return nc


def make_core_inputs(c, x, W_packed, b_packed, W_out):
    """Host-side shard prep for core c (pure numpy reshuffles)."""
    import ml_dtypes
    k, b = c % 4, c // 4
    heads = [12 + k, 8 + k, 4 + k, k]          # slots A..D
    rows = np.concatenate([np.arange(h * 64, (h + 1) * 64) for h in heads])

    xTc = np.ascontiguousarray(x[b].T)                      # [D, S]
    wq = W_packed[rows]                                     # [256, D]
    wk = W_packed[D + rows]
    wv = W_packed[2 * D + rows]
    wqkvT = np.ascontiguousarray(
        np.concatenate([wq, wk, wv], 0).T)                  # [D, 768]

    woTc = np.ascontiguousarray(W_out[:, rows].T)           # [256, D]
    woP = np.concatenate([woTc[:128], woTc[128:]], axis=1)  # [128, 2D]

    bq = b_packed[rows] / 8.0
    bk = b_packed[D + rows]
    bqk = np.stack([bq[:128], bq[128:], bk[:128], bk[128:]], 1)  # [128, 4]

    btabq = np.zeros((128, TABW + 4), np.float32)
    p = np.arange(128, dtype=np.float64)[:, None]
    for s in range(4):
        h = heads[s]
        slope = 2.0 ** (-(h + 1) * 8.0 / H)
        K, off0, tw, to = (SLOT_KEEP[s], SLOT_OFF0[s], SLOT_TABW[s],
                           SLOT_TABOFF[s])
        m = np.arange(tw, dtype=np.float64)[None, :]
        btabq[:, to:to + tw] = (slope * (p + 128.0 * (m - (K - 1)) - off0)
                                ).astype(np.float32)
    btabq[:, TABW:] = bqk.astype(np.float32)
    return heads, {"xT": xTc.astype(ml_dtypes.bfloat16),
                   "wqkvT": wqkvT.astype(ml_dtypes.bfloat16),
                   "woT": woP.astype(ml_dtypes.bfloat16),
                   "btabq": btabq}


_NC_CACHE = {}


def _get_program():
    if "nc" not in _NC_CACHE:
        _NC_CACHE["nc"] = build_program()
    return _NC_CACHE["nc"]


def make_in_maps(x, W_packed, b_packed, W_out):
    return [make_core_inputs(c, x, W_packed, b_packed, W_out)[1]
            for c in range(NCORES)]


def kernel(x, W_packed, b_packed, W_out, b_out):
    x = np.asarray(x, np.float32)
    W_packed = np.asarray(W_packed, np.float32)
    b_packed = np.asarray(b_packed, np.float32)
    W_out = np.asarray(W_out, np.float32)
    b_out = np.asarray(b_out, np.float32)

    nc = _get_program()
    in_maps = make_in_maps(x, W_packed, b_packed, W_out)
    res = run_bass_kernel_spmd(nc, in_maps, core_ids=list(range(NCORES)))

    # Gather: sum partials per batch; add b_out and the folded v-bias term.
    b_v = b_packed[2 * D:]
    bias_row = (b_out + W_out @ b_v).astype(np.float32)     # [D]
    full = np.empty((B, S, D), np.float32)
    for b in range(B):
        acc = res.results[4 * b]["out"].astype(np.float32).copy()
        for c in range(4 * b + 1, 4 * b + 4):
            acc += res.results[c]["out"]
        full[b] = acc + bias_row
    return full


# revision 54
# speedup vs baseline: 1.0674x; 1.0674x over previous
"""Trainium2 Bass kernel for causal multi-head attention with ALiBi.

Computes, for x:[B,S,D]:
    qkv = x @ W_packed.T + b_packed ; q,k,v = split(qkv)
    heads -> scores = q k^T / sqrt(hd) + alibi_causal_bias
    out = softmax(scores) v -> merge heads -> out @ W_out.T + b_out

Sharding (8 cores): core c handles batch c//4 and heads {k, k+4, k+8, k+12}
(k = c%4), one head per "slot". Slot block-schedules are head-independent
(sized for the largest ALiBi window in the slot), so one SPMD program runs
on all 8 cores; only the data (weight slices, bias tables) differs.
Host sums the 4 out-projection partials per batch and adds
b_out + W_out @ b_v (the v-bias term commutes through attention).

ALiBi sparsity: head h attends effectively only a bounded window back;
dropped softmax mass is <= e^-8 at the worst (group-start) columns.
Slots keep only the causal k-blocks within that window (KEEP blocks).

Softmax without row-max: scores are O(+-6), and exp is recentred per
q-group by C_g (a per-column constant that cancels in normalization),
keeping exp args in fp32 range. In the transposed layout scoresT[k,q] the
recentred ALiBi bias slope*(k - C_g) is per-partition, so it rides the
single Exp activation for free. Row sums come from a ones-row appended to
v in the attn@v matmul; normalization divides by that row.
"""

import os
import sys

import numpy as np

for _p in ("/opt/trn_rl_repo",):
    if os.path.isdir(_p) and _p not in sys.path:
        sys.path.append(_p)

import concourse.bacc as bacc
import concourse.bass as bass
import concourse.tile as tile
from concourse import mybir
from concourse.bass_utils import run_bass_kernel_spmd

B, S, D, H, HD = 2, 2048, 1024, 16, 64
NBLK = S // 128          # 16 k/q blocks
NCORES = 8

F32 = mybir.dt.float32
F32R = mybir.dt.float32r
BF16 = mybir.dt.bfloat16

# Slots A..D: per-core heads [12+k, 8+k, 4+k, k].  KEEP = causal k-blocks
# kept per q-block (max over the slot's heads).  W = q-group width.
SLOT_KEEP = (17, 8, 5, 2)
SLOT_W = (512, 512, 512, 128)
SLOT_OFF0 = (128, 128, 128, 64)
SLOT_TABW = tuple(k + 3 if w == 512 else k for k, w in zip(SLOT_KEEP, SLOT_W))
SLOT_TABOFF = tuple(int(np.cumsum((0,) + SLOT_TABW)[i]) for i in range(4))
TABW = int(sum(SLOT_TABW))


def _slot_schedule(s):
    """Yield (g, q0, W, [(j, lo, hi, tabcol, isdiag), ...]) per q-group."""
    K, W, _ = SLOT_KEEP[s], SLOT_W[s], SLOT_OFF0[s]
    out = []
    if W == 512:
        for g in range(S // 512):
            jlo = max(0, 4 * g + 3 - (K - 1))
            blocks = []
            for j in range(jlo, 4 * g + 4):
                lo = max(0, (j - 4 * g) * 128)
                hi = min(512, (j - 4 * g + K) * 128)
                m = j - 4 * g + (K - 1)
                blocks.append((j, lo, hi, SLOT_TABOFF[s] + m, j >= 4 * g))
            out.append((g, g * 512, 512, blocks))
    else:
        for i in range(NBLK):
            blocks = []
            for j in range(max(0, i - (K - 1)), i + 1):
                m = j - i + (K - 1)
                blocks.append((j, 0, 128, SLOT_TABOFF[s] + m, j == i))
            out.append((i, i * 128, 128, blocks))
    return out


def build_program():
    nc = bacc.Bacc("TRN2", target_bir_lowering=False, debug=False,
                   num_devices=NCORES)

    xT = nc.dram_tensor("xT", [D, S], BF16, kind="ExternalInput")
    wqkvT = nc.dram_tensor("wqkvT", [D, 768], BF16, kind="ExternalInput")
    woT = nc.dram_tensor("woT", [128, 2 * D], BF16, kind="ExternalInput")
    btabq = nc.dram_tensor("btabq", [128, TABW + 4], F32, kind="ExternalInput")
    out = nc.dram_tensor("out", [S, D], BF16, kind="ExternalOutput")

    with tile.TileContext(nc) as tc:
        with tc.tile_pool(name="persist", bufs=1) as pp:
            qkT = [pp.tile([128, S], BF16, tag=f"qkT{t}", name=f"qkT{t}")
                   for t in range(4)]
            v_t = pp.tile([128, 4, NBLK, 65], BF16, tag="v", name="v")
            hoT = [pp.tile([128, S], BF16, tag=f"hoT{t}", name=f"hoT{t}")
                   for t in range(2)]
            btab_sb = pp.tile([128, TABW + 4], F32, tag="btab", name="btab")
            ones_r = pp.tile([65, 64], F32, tag="ones_r", name="ones_r")
            wo_sb = pp.tile([128, 2 * D], BF16, tag="wo", name="wo")

            nc.gpsimd.memset(v_t[:, :, :, 64:65], 1.0)
            nc.gpsimd.memset(ones_r[:], 1.0)
            # touch the ACT engine once while it is idle so the activation
            # table load happens here, not on the first real Exp/Identity
            warm = pp.tile([1, 1], F32, tag="warm", name="warm")
            nc.scalar.activation(warm[:], ones_r[0:1, 0:1],
                                 mybir.ActivationFunctionType.Exp)

            # PSUM: 8 banks as 4 tags; phase-1 QKV borrows all four tags
            with (
                tc.tile_pool(name="xw", bufs=1) as xw,
                tc.tile_pool(name="et", bufs=8) as etp,
                tc.tile_pool(name="nrm", bufs=3) as nrm,
                tc.tile_pool(name="ob", bufs=2) as obp,
                tc.tile_pool(name="ps_sc", bufs=3, space="PSUM") as sc_ps,
                tc.tile_pool(name="ps_av", bufs=2, space="PSUM") as av_ps,
                tc.tile_pool(name="ps_bp", bufs=1, space="PSUM") as bp_ps,
                tc.tile_pool(name="ps_op", bufs=2, space="PSUM") as op_ps,
            ):
                # input stream: (wqkv_m, x_m) pairs first -- qkv phase 1
                # consumes chunk-major right behind the stream -- then the
                # small tables and the out-proj weight (needed much later).
                # weights ride the Pool DMA queue, x the SP queue: the two
                # streams overlap (aggregate stays under the HBM cap since
                # the weight stream is a third of the x stream)
                xT_sb, wqkv_sb = [], []
                for m in range(8):
                    t = xw.tile([128, 768], BF16, tag=f"wqkv{m}",
                                name=f"wqkv{m}")
                    nc.gpsimd.dma_start(t[:], wqkvT[m * 128:(m + 1) * 128, :])
                    wqkv_sb.append(t)
                    t = xw.tile([128, S], BF16, tag=f"x{m}", name=f"x{m}")
                    nc.sync.dma_start(t[:], xT[m * 128:(m + 1) * 128, :])
                    xT_sb.append(t)
                nc.gpsimd.dma_start(btab_sb[:], btabq[:])
                nc.gpsimd.dma_start(wo_sb[:], woT[:])

                def p1_tile(i, w):
                    pool, tag = [(sc_ps, "sc"), (sc_ps, "sc"), (sc_ps, "sc"),
                                 (bp_ps, "bps"), (av_ps, "av"), (av_ps, "av"),
                                 (op_ps, "op"), (op_ps, "op")][i]
                    return pool.tile([128, w], F32, tag=tag, name=f"p1_{i}")

                def qk_half(half):
                    # quarters {2h,2h+1} x 4 f-tiles -> 8 one-bank psums
                    # (m-outer: first matmul waits only for chunk-0 DMAs)
                    pss = {}
                    for ft in range(4):
                        for qi in range(2):
                            pss[ft, qi] = p1_tile(ft * 2 + qi, 512)
                    for m in range(8):
                        for ft in range(4):
                            for qi in range(2):
                                q4 = half * 2 + qi
                                nc.tensor.matmul(
                                    pss[ft, qi][:],
                                    wqkv_sb[m][:, ft * 128:(ft + 1) * 128],
                                    xT_sb[m][:, q4 * 512:(q4 + 1) * 512],
                                    start=(m == 0), stop=(m == 7),
                                )
                    # drain ft=3 first: its psums sit on the "op" tag the
                    # v/qk1 filler units need next.  ft=3 and qi=1 drains
                    # ride ACT (idle here) to shorten the DVE chain.
                    for ft in (3, 0, 1, 2):
                        for qi in range(2):
                            q4 = half * 2 + qi
                            scol = slice(q4 * 512, (q4 + 1) * 512)
                            # psum*scale + bias (1/sqrt(hd) folded into q)
                            if qi == 1:
                                nc.scalar.activation(
                                    qkT[ft][:, scol], pss[ft, qi][:],
                                    mybir.ActivationFunctionType.Identity,
                                    bias=btab_sb[:, TABW + ft:TABW + ft + 1],
                                    scale=(0.125 if ft < 2 else 1.0),
                                )
                            else:
                                nc.vector.tensor_scalar(
                                    out=qkT[ft][:, scol], in0=pss[ft, qi][:],
                                    scalar1=(0.125 if ft < 2 else 1.0),
                                    scalar2=btab_sb[:, TABW + ft:
                                                    TABW + ft + 1],
                                    op0=mybir.AluOpType.mult,
                                    op1=mybir.AluOpType.add,
                                )

                def v_unit(sb):
                    # one k-block of v for all 4 slots; m-inner, 1/2 bank
                    ps = op_ps.tile([128, 256], F32, tag="op", name="vps")
                    for m in range(8):
                        nc.tensor.matmul(
                            ps[:],
                            xT_sb[m][:, sb * 128:(sb + 1) * 128],
                            wqkv_sb[m][:, 512:768],
                            start=(m == 0), stop=(m == 7),
                        )
                    nc.vector.tensor_copy(
                        v_t[:, :, sb, 0:64],
                        ps[:].rearrange("p (s c) -> p s c", s=4),
                    )

                def qk1_unit(ft, qi):
                    # one (f-tile, q-quarter) of the second qk half; 1 bank
                    q4 = 2 + qi
                    ps = op_ps.tile([128, 512], F32, tag="op", name="qk1ps")
                    for m in range(8):
                        nc.tensor.matmul(
                            ps[:],
                            wqkv_sb[m][:, ft * 128:(ft + 1) * 128],
                            xT_sb[m][:, q4 * 512:(q4 + 1) * 512],
                            start=(m == 0), stop=(m == 7),
                        )
                    scol = slice(q4 * 512, (q4 + 1) * 512)
                    if ft >= 2:
                        nc.scalar.activation(
                            qkT[ft][:, scol], ps[:],
                            mybir.ActivationFunctionType.Identity,
                            bias=btab_sb[:, TABW + ft:TABW + ft + 1],
                            scale=1.0,
                        )
                    else:
                        nc.vector.tensor_scalar(
                            out=qkT[ft][:, scol], in0=ps[:],
                            scalar1=0.125,
                            scalar2=btab_sb[:, TABW + ft:TABW + ft + 1],
                            op0=mybir.AluOpType.mult,
                            op1=mybir.AluOpType.add,
                        )

                qk_half(0)
                for sb in range(4):
                    v_unit(sb)

                # filler units: PE work spliced into the exp-paced attention
                # stream.  Order respects data deps (g2 needs qk1 qi=0 and
                # v 8-11, g3 needs qi=1 and v 12-15; all are emitted at
                # least one full q-group before first use).
                from collections import deque
                fillers = deque()
                for sb in range(4, 8):
                    fillers.append(("v", sb))
                for ft in range(4):
                    fillers.append(("qk1", ft, 0))
                for sb in range(8, 12):
                    fillers.append(("v", sb))
                for ft in range(4):
                    fillers.append(("qk1", ft, 1))
                for sb in range(12, 16):
                    fillers.append(("v", sb))

                def emit_filler():
                    if not fillers:
                        return
                    u = fillers.popleft()
                    if u[0] == "v":
                        v_unit(u[1])
                    elif u[0] == "qk1":
                        qk1_unit(u[1], u[2])
                    else:
                        op_block(u[1])

                # ---- attention (+ out-proj interleaved per q-group) ----
                sched = [_slot_schedule(s) for s in range(4)]

                def scores_av(s, ent, av, coff):
                    """Scores+exp+AV for one q-group into av[:, coff:+W].

                    Generator, two yields per block: after the score matmul
                    and after the (previous block's) AV matmul.  Pairing two
                    slots' generators puts their 64-contraction score
                    matmuls back-to-back on disjoint PE row groups (slot
                    parity gives base partitions 0/64), so they execute
                    concurrently on HW.  The av-before-sc offset is also a
                    depth-1 software pipeline past the exp dependency."""
                    po = (s % 2) * 64
                    qT_s = qkT[s // 2][po:po + 64, :]
                    kT_s = qkT[2 + s // 2][po:po + 64, :]
                    g, q0, W, blocks = ent
                    pend = None
                    for bi, (j, lo, hi, tcol, isdiag) in enumerate(blocks):
                        # partial widths: block 0 always spans [0:W] (sets
                        # has_written on the full av range); later blocks
                        # touch only their causal window [lo:hi].
                        sc = sc_ps.tile([128, W], F32, tag="sc", name="sc")
                        nc.tensor.matmul(
                            sc[:, lo:hi],
                            kT_s[:, j * 128:(j + 1) * 128],
                            qT_s[:, q0 + lo:q0 + hi],
                        )
                        et = etp.tile([128, W], BF16, tag="et", name="et")
                        nc.scalar.activation(
                            et[:, lo:hi], sc[:, lo:hi],
                            mybir.ActivationFunctionType.Exp,
                            bias=btab_sb[:, tcol:tcol + 1], scale=1.0,
                        )
                        if isdiag:
                            # zero k>q inside the diagonal 128x128 block
                            nc.gpsimd.affine_select(
                                out=et[:, lo:lo + 128],
                                in_=et[:, lo:lo + 128],
                                compare_op=mybir.AluOpType.is_ge,
                                fill=0.0, base=0,
                                pattern=[[1, 128]],
                                channel_multiplier=-1,
                            )
                        yield
                        if pend is not None:
                            pbi, pj, plo, phi, pet = pend
                            nc.tensor.matmul(
                                av[:, coff + plo:coff + phi],
                                v_t[:, s, pj, :], pet[:, plo:phi],
                                start=(pbi == 0), stop=False,
                            )
                        pend = (bi, j, lo, hi, et)
                        yield
                    pbi, pj, plo, phi, pet = pend
                    nc.tensor.matmul(
                        av[:, coff + plo:coff + phi], v_t[:, s, pj, :],
                        pet[:, plo:phi], start=(pbi == 0), stop=True,
                    )

                def run_gens(gens, fill_every=6):
                    """Round-robin the slot generators; splice a filler
                    unit into the PE stream every `fill_every` steps."""
                    gens = list(gens)
                    steps = 0
                    while gens:
                        for gx in list(gens):
                            try:
                                next(gx)
                            except StopIteration:
                                gens.remove(gx)
                                continue
                            steps += 1
                            if steps % fill_every == 0:
                                emit_filler()

                def norm(s, av, q0, W):
                    """Divide av[0:64] by the ones-row sum; write hoT."""
                    po = (s % 2) * 64
                    hoT_s = hoT[s // 2]
                    lr = nrm.tile([65, W], F32R, tag="lr", name="lr")
                    nc.vector.tensor_copy(lr[64:65, :], av[64:65, :])
                    bps = bp_ps.tile([64, W], F32, tag="bps", name="bps")
                    nc.tensor.matmul(
                        bps[:], ones_r[64:65, 0:64].bitcast(F32R),
                        lr[64:65, :])
                    binv = nrm.tile([64, W], F32, tag="binv", name="binv")
                    nc.vector.reciprocal_approx_fast(out=binv[:], in_=bps[:])
                    if po == 0:
                        nc.vector.tensor_mul(
                            hoT_s[0:64, q0:q0 + W], av[0:64, :], binv[:])
                    else:
                        # DVE lanes can't shift partitions; bounce via DMA
                        tmp = nrm.tile([64, W], BF16, tag="hotmp",
                                       name="hotmp")
                        nc.vector.tensor_mul(tmp[:], av[0:64, :], binv[:])
                        # SP queue: keeps the Pool queue free for the
                        # diagonal-mask affine_selects
                        nc.sync.dma_start(
                            hoT_s[64:128, q0:q0 + W], tmp[:])

                def quad_d(g, avD):
                    for i4 in range(4):
                        yield from scores_av(3, sched[3][4 * g + i4], avD,
                                             i4 * 128)

                def op_block(sb):
                    ob = obp.tile([128, D], BF16, tag="ob", name="ob")
                    # cc-outer: consecutive matmuls share the stationary
                    # hoT slice (halves the LDWEIGHTS traffic on HW).
                    # Tail blocks alternate between the freed sc pool and
                    # the op pool so drains of consecutive blocks overlap.
                    pool = sc_ps if (sb >= 12 and sb % 2 == 0) else op_ps
                    tag = "sc" if pool is sc_ps else "op"
                    pss = [pool.tile([128, 512], F32, tag=tag, name="op")
                           for _ in range(2)]
                    for cc in range(2):
                        for jh in range(2):
                            nc.tensor.matmul(
                                pss[jh][:],
                                hoT[cc][:, sb * 128:(sb + 1) * 128],
                                wo_sb[:, cc * D + jh * 512:
                                      cc * D + (jh + 1) * 512],
                                start=(cc == 0), stop=(cc == 1),
                            )
                    if sb >= 12:
                        # tail: drain halves on ACT/DVE and ship each half
                        # immediately on its own DMA queue
                        nc.scalar.copy(ob[:, 0:512], pss[0][:])
                        nc.sync.dma_start(out[sb * 128:(sb + 1) * 128, 0:512],
                                          ob[:, 0:512])
                        nc.vector.tensor_copy(ob[:, 512:1024], pss[1][:])
                        nc.gpsimd.dma_start(
                            out[sb * 128:(sb + 1) * 128, 512:1024],
                            ob[:, 512:1024])
                    else:
                        for jh in range(2):
                            nc.vector.tensor_copy(
                                ob[:, jh * 512:(jh + 1) * 512], pss[jh][:])
                        nc.sync.dma_start(out[sb * 128:(sb + 1) * 128, :],
                                          ob[:])

                for g in range(4):
                    nfill = 1 if g < 3 else 2
                    # slot D first: its norm bounces hoT rows through a
                    # DMA whose latency then hides under slots A-C
                    avD = av_ps.tile([65, 512], F32, tag="av", name="avD")
                    for i4 in range(4):
                        run_gens([scores_av(3, sched[3][4 * g + i4], avD,
                                            i4 * 128)], 10**6)
                        for _ in range(nfill):
                            emit_filler()
                    norm(3, avD, g * 512, 512)
                    for s in (0, 1, 2):
                        avS = av_ps.tile([65, 512], F32, tag="av",
                                         name="avS")
                        run_gens([scores_av(s, sched[s][g], avS, 0)], 10**6)
                        norm(s, avS, g * 512, 512)
                        for _ in range(nfill):
                            emit_filler()
                    for sb in range(4 * g, 4 * g + 4):
                        fillers.append(("op", sb))
                while fillers:
                    emit_filler()

    nc.compile()
    return nc


def make_core_inputs(c, x, W_packed, b_packed, W_out):
    """Host-side shard prep for core c (pure numpy reshuffles)."""
    import ml_dtypes
    k, b = c % 4, c // 4
    heads = [12 + k, 8 + k, 4 + k, k]          # slots A..D
    rows = np.concatenate([np.arange(h * 64, (h + 1) * 64) for h in heads])

    xTc = np.ascontiguousarray(x[b].T)                      # [D, S]
    wq = W_packed[rows]                                     # [256, D]
    wk = W_packed[D + rows]
    wv = W_packed[2 * D + rows]
    wqkvT = np.ascontiguousarray(
        np.concatenate([wq, wk, wv], 0).T)                  # [D, 768]

    woTc = np.ascontiguousarray(W_out[:, rows].T)           # [256, D]
    woP = np.concatenate([woTc[:128], woTc[128:]], axis=1)  # [128, 2D]

    bq = b_packed[rows] / 8.0
    bk = b_packed[D + rows]
    bqk = np.stack([bq[:128], bq[128:], bk[:128], bk[128:]], 1)  # [128, 4]

    btabq = np.zeros((128, TABW + 4), np.float32)
    p = np.arange(128, dtype=np.float64)[:, None]
    for s in range(4):
        h = heads[s]
        slope = 2.0 ** (-(h + 1) * 8.0 / H)
        K, off0, tw, to = (SLOT_KEEP[s], SLOT_OFF0[s], SLOT_TABW[s],
                           SLOT_TABOFF[s])
        m = np.arange(tw, dtype=np.float64)[None, :]
        btabq[:, to:to + tw] = (slope * (p + 128.0 * (m - (K - 1)) - off0)
                                ).astype(np.float32)
    btabq[:, TABW:] = bqk.astype(np.float32)
    return heads, {"xT": xTc.astype(ml_dtypes.bfloat16),
                   "wqkvT": wqkvT.astype(ml_dtypes.bfloat16),
                   "woT": woP.astype(ml_dtypes.bfloat16),
                   "btabq": btabq}


_NC_CACHE = {}


def _get_program():
    if "nc" not in _NC_CACHE:
        _NC_CACHE["nc"] = build_program()
    return _NC_CACHE["nc"]


def make_in_maps(x, W_packed, b_packed, W_out):
    return [make_core_inputs(c, x, W_packed, b_packed, W_out)[1]
            for c in range(NCORES)]


def kernel(x, W_packed, b_packed, W_out, b_out):
    x = np.asarray(x, np.float32)
    W_packed = np.asarray(W_packed, np.float32)
    b_packed = np.asarray(b_packed, np.float32)
    W_out = np.asarray(W_out, np.float32)
    b_out = np.asarray(b_out, np.float32)

    nc = _get_program()
    in_maps = make_in_maps(x, W_packed, b_packed, W_out)
    res = run_bass_kernel_spmd(nc, in_maps, core_ids=list(range(NCORES)))

    # Gather: sum partials per batch; add b_out and the folded v-bias term.
    b_v = b_packed[2 * D:]
    bias_row = (b_out + W_out @ b_v).astype(np.float32)     # [D]
    full = np.empty((B, S, D), np.float32)
    for b in range(B):
        acc = res.results[4 * b]["out"].astype(np.float32).copy()
        for c in range(4 * b + 1, 4 * b + 4):
            acc += res.results[c]["out"]
        full[b] = acc + bias_row
    return full


# revision 61
# speedup vs baseline: 1.0739x; 1.0061x over previous
"""Trainium2 Bass kernel for causal multi-head attention with ALiBi.

Computes, for x:[B,S,D]:
    qkv = x @ W_packed.T + b_packed ; q,k,v = split(qkv)
    heads -> scores = q k^T / sqrt(hd) + alibi_causal_bias
    out = softmax(scores) v -> merge heads -> out @ W_out.T + b_out

Sharding (8 cores): core c handles batch c//4 and heads {k, k+4, k+8, k+12}
(k = c%4), one head per "slot". Slot block-schedules are head-independent
(sized for the largest ALiBi window in the slot), so one SPMD program runs
on all 8 cores; only the data (weight slices, bias tables) differs.
Host sums the 4 out-projection partials per batch and adds
b_out + W_out @ b_v (the v-bias term commutes through attention).

ALiBi sparsity: head h attends effectively only a bounded window back;
dropped softmax mass is <= e^-8 at the worst (group-start) columns.
Slots keep only the causal k-blocks within that window (KEEP blocks).

Softmax without row-max: scores are O(+-6), and exp is recentred per
q-group by C_g (a per-column constant that cancels in normalization),
keeping exp args in fp32 range. In the transposed layout scoresT[k,q] the
recentred ALiBi bias slope*(k - C_g) is per-partition, so it rides the
single Exp activation for free. Row sums come from a ones-row appended to
v in the attn@v matmul; normalization divides by that row.
"""

import os
import sys

import numpy as np

for _p in ("/opt/trn_rl_repo",):
    if os.path.isdir(_p) and _p not in sys.path:
        sys.path.append(_p)

import concourse.bacc as bacc
import concourse.bass as bass
import concourse.tile as tile
from concourse import mybir
from concourse.bass_utils import run_bass_kernel_spmd

B, S, D, H, HD = 2, 2048, 1024, 16, 64
NBLK = S // 128          # 16 k/q blocks
NCORES = 8

F32 = mybir.dt.float32
F32R = mybir.dt.float32r
BF16 = mybir.dt.bfloat16

# Slots A..D: per-core heads [12+k, 8+k, 4+k, k].  KEEP = causal k-blocks
# kept per q-block (max over the slot's heads).  W = q-group width.
SLOT_KEEP = (17, 7, 5, 2)
SLOT_W = (512, 512, 512, 128)
SLOT_OFF0 = (128, 128, 128, 64)
SLOT_TABW = tuple(k + 3 if w == 512 else k for k, w in zip(SLOT_KEEP, SLOT_W))
SLOT_TABOFF = tuple(int(np.cumsum((0,) + SLOT_TABW)[i]) for i in range(4))
TABW = int(sum(SLOT_TABW))


def _slot_schedule(s):
    """Yield (g, q0, W, [(j, lo, hi, tabcol, isdiag), ...]) per q-group."""
    K, W, _ = SLOT_KEEP[s], SLOT_W[s], SLOT_OFF0[s]
    out = []
    if W == 512:
        for g in range(S // 512):
            jlo = max(0, 4 * g + 3 - (K - 1))
            blocks = []
            for j in range(jlo, 4 * g + 4):
                lo = max(0, (j - 4 * g) * 128)
                hi = min(512, (j - 4 * g + K) * 128)
                m = j - 4 * g + (K - 1)
                blocks.append((j, lo, hi, SLOT_TABOFF[s] + m, j >= 4 * g))
            out.append((g, g * 512, 512, blocks))
    else:
        for i in range(NBLK):
            blocks = []
            for j in range(max(0, i - (K - 1)), i + 1):
                m = j - i + (K - 1)
                blocks.append((j, 0, 128, SLOT_TABOFF[s] + m, j == i))
            out.append((i, i * 128, 128, blocks))
    return out


def build_program():
    nc = bacc.Bacc("TRN2", target_bir_lowering=False, debug=False,
                   num_devices=NCORES)

    xT = nc.dram_tensor("xT", [D, S], BF16, kind="ExternalInput")
    wqkvT = nc.dram_tensor("wqkvT", [D, 768], BF16, kind="ExternalInput")
    woT = nc.dram_tensor("woT", [128, 2 * D], BF16, kind="ExternalInput")
    btabq = nc.dram_tensor("btabq", [128, TABW + 4], F32, kind="ExternalInput")
    out = nc.dram_tensor("out", [S, D], BF16, kind="ExternalOutput")

    with tile.TileContext(nc) as tc:
        with tc.tile_pool(name="persist", bufs=1) as pp:
            qkT = [pp.tile([128, S], BF16, tag=f"qkT{t}", name=f"qkT{t}")
                   for t in range(4)]
            v_t = pp.tile([128, 4, NBLK, 65], BF16, tag="v", name="v")
            hoT = [pp.tile([128, S], BF16, tag=f"hoT{t}", name=f"hoT{t}")
                   for t in range(2)]
            btab_sb = pp.tile([128, TABW + 4], F32, tag="btab", name="btab")
            ones_r = pp.tile([65, 64], F32, tag="ones_r", name="ones_r")
            wo_sb = pp.tile([128, 2 * D], BF16, tag="wo", name="wo")

            nc.gpsimd.memset(v_t[:, :, :, 64:65], 1.0)
            nc.gpsimd.memset(ones_r[:], 1.0)
            # touch the ACT engine once while it is idle so the activation
            # table load happens here, not on the first real Exp/Identity
            warm = pp.tile([1, 1], F32, tag="warm", name="warm")
            nc.scalar.activation(warm[:], ones_r[0:1, 0:1],
                                 mybir.ActivationFunctionType.Exp)

            # PSUM: 8 banks as 4 tags; phase-1 QKV borrows all four tags
            with (
                tc.tile_pool(name="xw", bufs=1) as xw,
                tc.tile_pool(name="et", bufs=8) as etp,
                tc.tile_pool(name="nrm", bufs=3) as nrm,
                tc.tile_pool(name="ob", bufs=2) as obp,
                tc.tile_pool(name="ps_sc", bufs=3, space="PSUM") as sc_ps,
                tc.tile_pool(name="ps_av", bufs=2, space="PSUM") as av_ps,
                tc.tile_pool(name="ps_bp", bufs=1, space="PSUM") as bp_ps,
                tc.tile_pool(name="ps_op", bufs=2, space="PSUM") as op_ps,
            ):
                # input stream: (wqkv_m, x_m) pairs first -- qkv phase 1
                # consumes chunk-major right behind the stream -- then the
                # small tables and the out-proj weight (needed much later).
                # weights ride the Pool DMA queue, x the SP queue: the two
                # streams overlap (aggregate stays under the HBM cap since
                # the weight stream is a third of the x stream)
                xT_sb, wqkv_sb = [], []
                for m in range(8):
                    t = xw.tile([128, 768], BF16, tag=f"wqkv{m}",
                                name=f"wqkv{m}")
                    nc.gpsimd.dma_start(t[:], wqkvT[m * 128:(m + 1) * 128, :])
                    wqkv_sb.append(t)
                    t = xw.tile([128, S], BF16, tag=f"x{m}", name=f"x{m}")
                    nc.sync.dma_start(t[:], xT[m * 128:(m + 1) * 128, :])
                    xT_sb.append(t)
                nc.gpsimd.dma_start(btab_sb[:], btabq[:])
                nc.gpsimd.dma_start(wo_sb[:], woT[:])

                def p1_tile(i, w):
                    pool, tag = [(sc_ps, "sc"), (sc_ps, "sc"), (sc_ps, "sc"),
                                 (bp_ps, "bps"), (av_ps, "av"), (av_ps, "av"),
                                 (op_ps, "op"), (op_ps, "op")][i]
                    return pool.tile([128, w], F32, tag=tag, name=f"p1_{i}")

                def qk_half(half):
                    # quarters {2h,2h+1} x 4 f-tiles -> 8 one-bank psums
                    # (m-outer: first matmul waits only for chunk-0 DMAs)
                    pss = {}
                    for ft in range(4):
                        for qi in range(2):
                            pss[ft, qi] = p1_tile(ft * 2 + qi, 512)
                    for m in range(8):
                        for ft in range(4):
                            for qi in range(2):
                                q4 = half * 2 + qi
                                nc.tensor.matmul(
                                    pss[ft, qi][:],
                                    wqkv_sb[m][:, ft * 128:(ft + 1) * 128],
                                    xT_sb[m][:, q4 * 512:(q4 + 1) * 512],
                                    start=(m == 0), stop=(m == 7),
                                )
                    # drain ft=3 first: its psums sit on the "op" tag the
                    # v/qk1 filler units need next.  ft=3 and qi=1 drains
                    # ride ACT (idle here) to shorten the DVE chain.
                    for ft in (3, 0, 1, 2):
                        for qi in range(2):
                            q4 = half * 2 + qi
                            scol = slice(q4 * 512, (q4 + 1) * 512)
                            # psum*scale + bias (1/sqrt(hd) folded into q)
                            if qi == 1:
                                nc.scalar.activation(
                                    qkT[ft][:, scol], pss[ft, qi][:],
                                    mybir.ActivationFunctionType.Identity,
                                    bias=btab_sb[:, TABW + ft:TABW + ft + 1],
                                    scale=(0.125 if ft < 2 else 1.0),
                                )
                            else:
                                nc.vector.tensor_scalar(
                                    out=qkT[ft][:, scol], in0=pss[ft, qi][:],
                                    scalar1=(0.125 if ft < 2 else 1.0),
                                    scalar2=btab_sb[:, TABW + ft:
                                                    TABW + ft + 1],
                                    op0=mybir.AluOpType.mult,
                                    op1=mybir.AluOpType.add,
                                )

                def v_unit(sb):
                    # one k-block of v for all 4 slots; m-inner, 1/2 bank
                    ps = op_ps.tile([128, 256], F32, tag="op", name="vps")
                    for m in range(8):
                        nc.tensor.matmul(
                            ps[:],
                            xT_sb[m][:, sb * 128:(sb + 1) * 128],
                            wqkv_sb[m][:, 512:768],
                            start=(m == 0), stop=(m == 7),
                        )
                    nc.vector.tensor_copy(
                        v_t[:, :, sb, 0:64],
                        ps[:].rearrange("p (s c) -> p s c", s=4),
                    )

                def qk1_unit(ft, qi):
                    # one (f-tile, q-quarter) of the second qk half; 1 bank
                    q4 = 2 + qi
                    ps = op_ps.tile([128, 512], F32, tag="op", name="qk1ps")
                    for m in range(8):
                        nc.tensor.matmul(
                            ps[:],
                            wqkv_sb[m][:, ft * 128:(ft + 1) * 128],
                            xT_sb[m][:, q4 * 512:(q4 + 1) * 512],
                            start=(m == 0), stop=(m == 7),
                        )
                    scol = slice(q4 * 512, (q4 + 1) * 512)
                    if ft >= 2:
                        nc.scalar.activation(
                            qkT[ft][:, scol], ps[:],
                            mybir.ActivationFunctionType.Identity,
                            bias=btab_sb[:, TABW + ft:TABW + ft + 1],
                            scale=1.0,
                        )
                    else:
                        nc.vector.tensor_scalar(
                            out=qkT[ft][:, scol], in0=ps[:],
                            scalar1=0.125,
                            scalar2=btab_sb[:, TABW + ft:TABW + ft + 1],
                            op0=mybir.AluOpType.mult,
                            op1=mybir.AluOpType.add,
                        )

                qk_half(0)
                for sb in range(4):
                    v_unit(sb)

                # filler units: PE work spliced into the exp-paced attention
                # stream.  Order respects data deps (g2 needs qk1 qi=0 and
                # v 8-11, g3 needs qi=1 and v 12-15; all are emitted at
                # least one full q-group before first use).
                from collections import deque
                fillers = deque()
                for sb in range(4, 8):
                    fillers.append(("v", sb))
                for ft in range(4):
                    fillers.append(("qk1", ft, 0))
                for sb in range(8, 12):
                    fillers.append(("v", sb))
                for ft in range(4):
                    fillers.append(("qk1", ft, 1))
                for sb in range(12, 16):
                    fillers.append(("v", sb))

                def emit_filler():
                    if not fillers:
                        return
                    u = fillers.popleft()
                    if u[0] == "v":
                        v_unit(u[1])
                    elif u[0] == "qk1":
                        qk1_unit(u[1], u[2])
                    else:
                        op_block(u[1])

                # ---- attention (+ out-proj interleaved per q-group) ----
                sched = [_slot_schedule(s) for s in range(4)]

                def scores_av(s, ent, av, coff):
                    """Scores+exp+AV for one q-group into av[:, coff:+W].

                    Generator, two yields per block: after the score matmul
                    and after the (previous block's) AV matmul.  Pairing two
                    slots' generators puts their 64-contraction score
                    matmuls back-to-back on disjoint PE row groups (slot
                    parity gives base partitions 0/64), so they execute
                    concurrently on HW.  The av-before-sc offset is also a
                    depth-1 software pipeline past the exp dependency."""
                    po = (s % 2) * 64
                    qT_s = qkT[s // 2][po:po + 64, :]
                    kT_s = qkT[2 + s // 2][po:po + 64, :]
                    g, q0, W, blocks = ent
                    pend = None
                    for bi, (j, lo, hi, tcol, isdiag) in enumerate(blocks):
                        # partial widths: block 0 always spans [0:W] (sets
                        # has_written on the full av range); later blocks
                        # touch only their causal window [lo:hi].
                        sc = sc_ps.tile([128, W], F32, tag="sc", name="sc")
                        nc.tensor.matmul(
                            sc[:, lo:hi],
                            kT_s[:, j * 128:(j + 1) * 128],
                            qT_s[:, q0 + lo:q0 + hi],
                        )
                        et = etp.tile([128, W], BF16, tag="et", name="et")
                        nc.scalar.activation(
                            et[:, lo:hi], sc[:, lo:hi],
                            mybir.ActivationFunctionType.Exp,
                            bias=btab_sb[:, tcol:tcol + 1], scale=1.0,
                        )
                        if isdiag:
                            # zero k>q inside the diagonal 128x128 block
                            nc.gpsimd.affine_select(
                                out=et[:, lo:lo + 128],
                                in_=et[:, lo:lo + 128],
                                compare_op=mybir.AluOpType.is_ge,
                                fill=0.0, base=0,
                                pattern=[[1, 128]],
                                channel_multiplier=-1,
                            )
                        yield
                        if pend is not None:
                            pbi, pj, plo, phi, pet = pend
                            nc.tensor.matmul(
                                av[:, coff + plo:coff + phi],
                                v_t[:, s, pj, :], pet[:, plo:phi],
                                start=(pbi == 0), stop=False,
                            )
                        pend = (bi, j, lo, hi, et)
                        yield
                    pbi, pj, plo, phi, pet = pend
                    nc.tensor.matmul(
                        av[:, coff + plo:coff + phi], v_t[:, s, pj, :],
                        pet[:, plo:phi], start=(pbi == 0), stop=True,
                    )

                def run_gens(gens, fill_every=6):
                    """Round-robin the slot generators; splice a filler
                    unit into the PE stream every `fill_every` steps."""
                    gens = list(gens)
                    steps = 0
                    while gens:
                        for gx in list(gens):
                            try:
                                next(gx)
                            except StopIteration:
                                gens.remove(gx)
                                continue
                            steps += 1
                            if steps % fill_every == 0:
                                emit_filler()

                def norm(s, av, q0, W):
                    """Divide av[0:64] by the ones-row sum; write hoT."""
                    po = (s % 2) * 64
                    hoT_s = hoT[s // 2]
                    lr = nrm.tile([65, W], F32R, tag="lr", name="lr")
                    nc.vector.tensor_copy(lr[64:65, :], av[64:65, :])
                    bps = bp_ps.tile([64, W], F32, tag="bps", name="bps")
                    nc.tensor.matmul(
                        bps[:], ones_r[64:65, 0:64].bitcast(F32R),
                        lr[64:65, :])
                    binv = nrm.tile([64, W], F32, tag="binv", name="binv")
                    nc.vector.reciprocal_approx_fast(out=binv[:], in_=bps[:])
                    if po == 0:
                        nc.vector.tensor_mul(
                            hoT_s[0:64, q0:q0 + W], av[0:64, :], binv[:])
                    else:
                        # DVE lanes can't shift partitions; bounce via DMA
                        tmp = nrm.tile([64, W], BF16, tag="hotmp",
                                       name="hotmp")
                        nc.vector.tensor_mul(tmp[:], av[0:64, :], binv[:])
                        # SP queue: keeps the Pool queue free for the
                        # diagonal-mask affine_selects
                        nc.sync.dma_start(
                            hoT_s[64:128, q0:q0 + W], tmp[:])

                def quad_d(g, avD):
                    for i4 in range(4):
                        yield from scores_av(3, sched[3][4 * g + i4], avD,
                                             i4 * 128)

                def op_block(sb):
                    ob = obp.tile([128, D], BF16, tag="ob", name="ob")
                    # cc-outer: consecutive matmuls share the stationary
                    # hoT slice (halves the LDWEIGHTS traffic on HW).
                    # Tail blocks alternate between the freed sc pool and
                    # the op pool so drains of consecutive blocks overlap.
                    pool = sc_ps if (sb >= 12 and sb % 2 == 0) else op_ps
                    tag = "sc" if pool is sc_ps else "op"
                    pss = [pool.tile([128, 512], F32, tag=tag, name="op")
                           for _ in range(2)]
                    for cc in range(2):
                        for jh in range(2):
                            nc.tensor.matmul(
                                pss[jh][:],
                                hoT[cc][:, sb * 128:(sb + 1) * 128],
                                wo_sb[:, cc * D + jh * 512:
                                      cc * D + (jh + 1) * 512],
                                start=(cc == 0), stop=(cc == 1),
                            )
                    if sb >= 12:
                        # tail: drain halves on ACT/DVE and ship each half
                        # immediately on its own DMA queue
                        nc.scalar.copy(ob[:, 0:512], pss[0][:])
                        nc.sync.dma_start(out[sb * 128:(sb + 1) * 128, 0:512],
                                          ob[:, 0:512])
                        nc.vector.tensor_copy(ob[:, 512:1024], pss[1][:])
                        nc.gpsimd.dma_start(
                            out[sb * 128:(sb + 1) * 128, 512:1024],
                            ob[:, 512:1024])
                    else:
                        for jh in range(2):
                            nc.vector.tensor_copy(
                                ob[:, jh * 512:(jh + 1) * 512], pss[jh][:])
                        nc.sync.dma_start(out[sb * 128:(sb + 1) * 128, :],
                                          ob[:])

                for g in range(4):
                    nfill = 1 if g < 3 else 2
                    # slot D first: its norm bounces hoT rows through a
                    # DMA whose latency then hides under slots A-C
                    avD = av_ps.tile([65, 512], F32, tag="av", name="avD")
                    for i4 in range(4):
                        run_gens([scores_av(3, sched[3][4 * g + i4], avD,
                                            i4 * 128)], 10**6)
                        for _ in range(nfill):
                            emit_filler()
                    norm(3, avD, g * 512, 512)
                    for s in (0, 1, 2):
                        avS = av_ps.tile([65, 512], F32, tag="av",
                                         name="avS")
                        run_gens([scores_av(s, sched[s][g], avS, 0)], 10**6)
                        norm(s, avS, g * 512, 512)
                        for _ in range(nfill):
                            emit_filler()
                    for sb in range(4 * g, 4 * g + 4):
                        fillers.append(("op", sb))
                while fillers:
                    emit_filler()

    nc.compile()
    return nc


def make_core_inputs(c, x, W_packed, b_packed, W_out):
    """Host-side shard prep for core c (pure numpy reshuffles)."""
    import ml_dtypes
    k, b = c % 4, c // 4
    heads = [12 + k, 8 + k, 4 + k, k]          # slots A..D
    rows = np.concatenate([np.arange(h * 64, (h + 1) * 64) for h in heads])

    xTc = np.ascontiguousarray(x[b].T)                      # [D, S]
    wq = W_packed[rows]                                     # [256, D]
    wk = W_packed[D + rows]
    wv = W_packed[2 * D + rows]
    wqkvT = np.ascontiguousarray(
        np.concatenate([wq, wk, wv], 0).T)                  # [D, 768]

    woTc = np.ascontiguousarray(W_out[:, rows].T)           # [256, D]
    woP = np.concatenate([woTc[:128], woTc[128:]], axis=1)  # [128, 2D]

    bq = b_packed[rows] / 8.0
    bk = b_packed[D + rows]
    bqk = np.stack([bq[:128], bq[128:], bk[:128], bk[128:]], 1)  # [128, 4]

    btabq = np.zeros((128, TABW + 4), np.float32)
    p = np.arange(128, dtype=np.float64)[:, None]
    for s in range(4):
        h = heads[s]
        slope = 2.0 ** (-(h + 1) * 8.0 / H)
        K, off0, tw, to = (SLOT_KEEP[s], SLOT_OFF0[s], SLOT_TABW[s],
                           SLOT_TABOFF[s])
        m = np.arange(tw, dtype=np.float64)[None, :]
        btabq[:, to:to + tw] = (slope * (p + 128.0 * (m - (K - 1)) - off0)
                                ).astype(np.float32)
    btabq[:, TABW:] = bqk.astype(np.float32)
    return heads, {"xT": xTc.astype(ml_dtypes.bfloat16),
                   "wqkvT": wqkvT.astype(ml_dtypes.bfloat16),
                   "woT": woP.astype(ml_dtypes.bfloat16),
                   "btabq": btabq}


_NC_CACHE = {}


def _get_program():
    if "nc" not in _NC_CACHE:
        _NC_CACHE["nc"] = build_program()
    return _NC_CACHE["nc"]


def make_in_maps(x, W_packed, b_packed, W_out):
    return [make_core_inputs(c, x, W_packed, b_packed, W_out)[1]
            for c in range(NCORES)]


def kernel(x, W_packed, b_packed, W_out, b_out):
    x = np.asarray(x, np.float32)
    W_packed = np.asarray(W_packed, np.float32)
    b_packed = np.asarray(b_packed, np.float32)
    W_out = np.asarray(W_out, np.float32)
    b_out = np.asarray(b_out, np.float32)

    nc = _get_program()
    in_maps = make_in_maps(x, W_packed, b_packed, W_out)
    res = run_bass_kernel_spmd(nc, in_maps, core_ids=list(range(NCORES)))

    # Gather: sum partials per batch; add b_out and the folded v-bias term.
    b_v = b_packed[2 * D:]
    bias_row = (b_out + W_out @ b_v).astype(np.float32)     # [D]
    full = np.empty((B, S, D), np.float32)
    for b in range(B):
        acc = res.results[4 * b]["out"].astype(np.float32).copy()
        for c in range(4 * b + 1, 4 * b + 4):
            acc += res.results[c]["out"]
        full[b] = acc + bias_row
    return full


# revision 62
# speedup vs baseline: 1.0784x; 1.0043x over previous
"""Trainium2 Bass kernel for causal multi-head attention with ALiBi.

Computes, for x:[B,S,D]:
    qkv = x @ W_packed.T + b_packed ; q,k,v = split(qkv)
    heads -> scores = q k^T / sqrt(hd) + alibi_causal_bias
    out = softmax(scores) v -> merge heads -> out @ W_out.T + b_out

Sharding (8 cores): core c handles batch c//4 and heads {k, k+4, k+8, k+12}
(k = c%4), one head per "slot". Slot block-schedules are head-independent
(sized for the largest ALiBi window in the slot), so one SPMD program runs
on all 8 cores; only the data (weight slices, bias tables) differs.
Host sums the 4 out-projection partials per batch and adds
b_out + W_out @ b_v (the v-bias term commutes through attention).

ALiBi sparsity: head h attends effectively only a bounded window back;
dropped softmax mass is <= e^-8 at the worst (group-start) columns.
Slots keep only the causal k-blocks within that window (KEEP blocks).

Softmax without row-max: scores are O(+-6), and exp is recentred per
q-group by C_g (a per-column constant that cancels in normalization),
keeping exp args in fp32 range. In the transposed layout scoresT[k,q] the
recentred ALiBi bias slope*(k - C_g) is per-partition, so it rides the
single Exp activation for free. Row sums come from a ones-row appended to
v in the attn@v matmul; normalization divides by that row.
"""

import os
import sys

import numpy as np

for _p in ("/opt/trn_rl_repo",):
    if os.path.isdir(_p) and _p not in sys.path:
        sys.path.append(_p)

import concourse.bacc as bacc
import concourse.bass as bass
import concourse.tile as tile
from concourse import mybir
from concourse.bass_utils import run_bass_kernel_spmd

B, S, D, H, HD = 2, 2048, 1024, 16, 64
NBLK = S // 128          # 16 k/q blocks
NCORES = 8

F32 = mybir.dt.float32
F32R = mybir.dt.float32r
BF16 = mybir.dt.bfloat16

# Slots A..D: per-core heads [12+k, 8+k, 4+k, k].  KEEP = causal k-blocks
# kept per q-block (max over the slot's heads).  W = q-group width.
SLOT_KEEP = (17, 6, 5, 2)
SLOT_W = (512, 512, 512, 128)
SLOT_OFF0 = (128, 128, 128, 64)
SLOT_TABW = tuple(k + 3 if w == 512 else k for k, w in zip(SLOT_KEEP, SLOT_W))
SLOT_TABOFF = tuple(int(np.cumsum((0,) + SLOT_TABW)[i]) for i in range(4))
TABW = int(sum(SLOT_TABW))


def _slot_schedule(s):
    """Yield (g, q0, W, [(j, lo, hi, tabcol, isdiag), ...]) per q-group."""
    K, W, _ = SLOT_KEEP[s], SLOT_W[s], SLOT_OFF0[s]
    out = []
    if W == 512:
        for g in range(S // 512):
            jlo = max(0, 4 * g + 3 - (K - 1))
            blocks = []
            for j in range(jlo, 4 * g + 4):
                lo = max(0, (j - 4 * g) * 128)
                hi = min(512, (j - 4 * g + K) * 128)
                m = j - 4 * g + (K - 1)
                blocks.append((j, lo, hi, SLOT_TABOFF[s] + m, j >= 4 * g))
            out.append((g, g * 512, 512, blocks))
    else:
        for i in range(NBLK):
            blocks = []
            for j in range(max(0, i - (K - 1)), i + 1):
                m = j - i + (K - 1)
                blocks.append((j, 0, 128, SLOT_TABOFF[s] + m, j == i))
            out.append((i, i * 128, 128, blocks))
    return out


def build_program():
    nc = bacc.Bacc("TRN2", target_bir_lowering=False, debug=False,
                   num_devices=NCORES)

    xT = nc.dram_tensor("xT", [D, S], BF16, kind="ExternalInput")
    wqkvT = nc.dram_tensor("wqkvT", [D, 768], BF16, kind="ExternalInput")
    woT = nc.dram_tensor("woT", [128, 2 * D], BF16, kind="ExternalInput")
    btabq = nc.dram_tensor("btabq", [128, TABW + 4], F32, kind="ExternalInput")
    out = nc.dram_tensor("out", [S, D], BF16, kind="ExternalOutput")

    with tile.TileContext(nc) as tc:
        with tc.tile_pool(name="persist", bufs=1) as pp:
            qkT = [pp.tile([128, S], BF16, tag=f"qkT{t}", name=f"qkT{t}")
                   for t in range(4)]
            v_t = pp.tile([128, 4, NBLK, 65], BF16, tag="v", name="v")
            hoT = [pp.tile([128, S], BF16, tag=f"hoT{t}", name=f"hoT{t}")
                   for t in range(2)]
            btab_sb = pp.tile([128, TABW + 4], F32, tag="btab", name="btab")
            ones_r = pp.tile([65, 64], F32, tag="ones_r", name="ones_r")
            wo_sb = pp.tile([128, 2 * D], BF16, tag="wo", name="wo")

            nc.gpsimd.memset(v_t[:, :, :, 64:65], 1.0)
            nc.gpsimd.memset(ones_r[:], 1.0)
            # touch the ACT engine once while it is idle so the activation
            # table load happens here, not on the first real Exp/Identity
            warm = pp.tile([1, 1], F32, tag="warm", name="warm")
            nc.scalar.activation(warm[:], ones_r[0:1, 0:1],
                                 mybir.ActivationFunctionType.Exp)

            # PSUM: 8 banks as 4 tags; phase-1 QKV borrows all four tags
            with (
                tc.tile_pool(name="xw", bufs=1) as xw,
                tc.tile_pool(name="et", bufs=8) as etp,
                tc.tile_pool(name="nrm", bufs=3) as nrm,
                tc.tile_pool(name="ob", bufs=2) as obp,
                tc.tile_pool(name="ps_sc", bufs=3, space="PSUM") as sc_ps,
                tc.tile_pool(name="ps_av", bufs=2, space="PSUM") as av_ps,
                tc.tile_pool(name="ps_bp", bufs=1, space="PSUM") as bp_ps,
                tc.tile_pool(name="ps_op", bufs=2, space="PSUM") as op_ps,
            ):
                # input stream: (wqkv_m, x_m) pairs first -- qkv phase 1
                # consumes chunk-major right behind the stream -- then the
                # small tables and the out-proj weight (needed much later).
                # weights ride the Pool DMA queue, x the SP queue: the two
                # streams overlap (aggregate stays under the HBM cap since
                # the weight stream is a third of the x stream)
                xT_sb, wqkv_sb = [], []
                for m in range(8):
                    t = xw.tile([128, 768], BF16, tag=f"wqkv{m}",
                                name=f"wqkv{m}")
                    nc.gpsimd.dma_start(t[:], wqkvT[m * 128:(m + 1) * 128, :])
                    wqkv_sb.append(t)
                    t = xw.tile([128, S], BF16, tag=f"x{m}", name=f"x{m}")
                    nc.sync.dma_start(t[:], xT[m * 128:(m + 1) * 128, :])
                    xT_sb.append(t)
                nc.gpsimd.dma_start(btab_sb[:], btabq[:])
                nc.gpsimd.dma_start(wo_sb[:], woT[:])

                def p1_tile(i, w):
                    pool, tag = [(sc_ps, "sc"), (sc_ps, "sc"), (sc_ps, "sc"),
                                 (bp_ps, "bps"), (av_ps, "av"), (av_ps, "av"),
                                 (op_ps, "op"), (op_ps, "op")][i]
                    return pool.tile([128, w], F32, tag=tag, name=f"p1_{i}")

                def qk_half(half):
                    # quarters {2h,2h+1} x 4 f-tiles -> 8 one-bank psums
                    # (m-outer: first matmul waits only for chunk-0 DMAs)
                    pss = {}
                    for ft in range(4):
                        for qi in range(2):
                            pss[ft, qi] = p1_tile(ft * 2 + qi, 512)
                    for m in range(8):
                        for ft in range(4):
                            for qi in range(2):
                                q4 = half * 2 + qi
                                nc.tensor.matmul(
                                    pss[ft, qi][:],
                                    wqkv_sb[m][:, ft * 128:(ft + 1) * 128],
                                    xT_sb[m][:, q4 * 512:(q4 + 1) * 512],
                                    start=(m == 0), stop=(m == 7),
                                )
                    # drain ft=3 first: its psums sit on the "op" tag the
                    # v/qk1 filler units need next.  ft=3 and qi=1 drains
                    # ride ACT (idle here) to shorten the DVE chain.
                    for ft in (3, 0, 1, 2):
                        for qi in range(2):
                            q4 = half * 2 + qi
                            scol = slice(q4 * 512, (q4 + 1) * 512)
                            # psum*scale + bias (1/sqrt(hd) folded into q)
                            if qi == 1:
                                nc.scalar.activation(
                                    qkT[ft][:, scol], pss[ft, qi][:],
                                    mybir.ActivationFunctionType.Identity,
                                    bias=btab_sb[:, TABW + ft:TABW + ft + 1],
                                    scale=(0.125 if ft < 2 else 1.0),
                                )
                            else:
                                nc.vector.tensor_scalar(
                                    out=qkT[ft][:, scol], in0=pss[ft, qi][:],
                                    scalar1=(0.125 if ft < 2 else 1.0),
                                    scalar2=btab_sb[:, TABW + ft:
                                                    TABW + ft + 1],
                                    op0=mybir.AluOpType.mult,
                                    op1=mybir.AluOpType.add,
                                )

                def v_unit(sb):
                    # one k-block of v for all 4 slots; m-inner, 1/2 bank
                    ps = op_ps.tile([128, 256], F32, tag="op", name="vps")
                    for m in range(8):
                        nc.tensor.matmul(
                            ps[:],
                            xT_sb[m][:, sb * 128:(sb + 1) * 128],
                            wqkv_sb[m][:, 512:768],
                            start=(m == 0), stop=(m == 7),
                        )
                    nc.vector.tensor_copy(
                        v_t[:, :, sb, 0:64],
                        ps[:].rearrange("p (s c) -> p s c", s=4),
                    )

                def qk1_unit(ft, qi):
                    # one (f-tile, q-quarter) of the second qk half; 1 bank
                    q4 = 2 + qi
                    ps = op_ps.tile([128, 512], F32, tag="op", name="qk1ps")
                    for m in range(8):
                        nc.tensor.matmul(
                            ps[:],
                            wqkv_sb[m][:, ft * 128:(ft + 1) * 128],
                            xT_sb[m][:, q4 * 512:(q4 + 1) * 512],
                            start=(m == 0), stop=(m == 7),
                        )
                    scol = slice(q4 * 512, (q4 + 1) * 512)
                    if ft >= 2:
                        nc.scalar.activation(
                            qkT[ft][:, scol], ps[:],
                            mybir.ActivationFunctionType.Identity,
                            bias=btab_sb[:, TABW + ft:TABW + ft + 1],
                            scale=1.0,
                        )
                    else:
                        nc.vector.tensor_scalar(
                            out=qkT[ft][:, scol], in0=ps[:],
                            scalar1=0.125,
                            scalar2=btab_sb[:, TABW + ft:TABW + ft + 1],
                            op0=mybir.AluOpType.mult,
                            op1=mybir.AluOpType.add,
                        )

                qk_half(0)
                for sb in range(4):
                    v_unit(sb)

                # filler units: PE work spliced into the exp-paced attention
                # stream.  Order respects data deps (g2 needs qk1 qi=0 and
                # v 8-11, g3 needs qi=1 and v 12-15; all are emitted at
                # least one full q-group before first use).
                from collections import deque
                fillers = deque()
                for sb in range(4, 8):
                    fillers.append(("v", sb))
                for ft in range(4):
                    fillers.append(("qk1", ft, 0))
                for sb in range(8, 12):
                    fillers.append(("v", sb))
                for ft in range(4):
                    fillers.append(("qk1", ft, 1))
                for sb in range(12, 16):
                    fillers.append(("v", sb))

                def emit_filler():
                    if not fillers:
                        return
                    u = fillers.popleft()
                    if u[0] == "v":
                        v_unit(u[1])
                    elif u[0] == "qk1":
                        qk1_unit(u[1], u[2])
                    else:
                        op_block(u[1])

                # ---- attention (+ out-proj interleaved per q-group) ----
                sched = [_slot_schedule(s) for s in range(4)]

                def scores_av(s, ent, av, coff):
                    """Scores+exp+AV for one q-group into av[:, coff:+W].

                    Generator, two yields per block: after the score matmul
                    and after the (previous block's) AV matmul.  Pairing two
                    slots' generators puts their 64-contraction score
                    matmuls back-to-back on disjoint PE row groups (slot
                    parity gives base partitions 0/64), so they execute
                    concurrently on HW.  The av-before-sc offset is also a
                    depth-1 software pipeline past the exp dependency."""
                    po = (s % 2) * 64
                    qT_s = qkT[s // 2][po:po + 64, :]
                    kT_s = qkT[2 + s // 2][po:po + 64, :]
                    g, q0, W, blocks = ent
                    pend = None
                    for bi, (j, lo, hi, tcol, isdiag) in enumerate(blocks):
                        # partial widths: block 0 always spans [0:W] (sets
                        # has_written on the full av range); later blocks
                        # touch only their causal window [lo:hi].
                        sc = sc_ps.tile([128, W], F32, tag="sc", name="sc")
                        nc.tensor.matmul(
                            sc[:, lo:hi],
                            kT_s[:, j * 128:(j + 1) * 128],
                            qT_s[:, q0 + lo:q0 + hi],
                        )
                        et = etp.tile([128, W], BF16, tag="et", name="et")
                        nc.scalar.activation(
                            et[:, lo:hi], sc[:, lo:hi],
                            mybir.ActivationFunctionType.Exp,
                            bias=btab_sb[:, tcol:tcol + 1], scale=1.0,
                        )
                        if isdiag:
                            # zero k>q inside the diagonal 128x128 block
                            nc.gpsimd.affine_select(
                                out=et[:, lo:lo + 128],
                                in_=et[:, lo:lo + 128],
                                compare_op=mybir.AluOpType.is_ge,
                                fill=0.0, base=0,
                                pattern=[[1, 128]],
                                channel_multiplier=-1,
                            )
                        yield
                        if pend is not None:
                            pbi, pj, plo, phi, pet = pend
                            nc.tensor.matmul(
                                av[:, coff + plo:coff + phi],
                                v_t[:, s, pj, :], pet[:, plo:phi],
                                start=(pbi == 0), stop=False,
                            )
                        pend = (bi, j, lo, hi, et)
                        yield
                    pbi, pj, plo, phi, pet = pend
                    nc.tensor.matmul(
                        av[:, coff + plo:coff + phi], v_t[:, s, pj, :],
                        pet[:, plo:phi], start=(pbi == 0), stop=True,
                    )

                def run_gens(gens, fill_every=6):
                    """Round-robin the slot generators; splice a filler
                    unit into the PE stream every `fill_every` steps."""
                    gens = list(gens)
                    steps = 0
                    while gens:
                        for gx in list(gens):
                            try:
                                next(gx)
                            except StopIteration:
                                gens.remove(gx)
                                continue
                            steps += 1
                            if steps % fill_every == 0:
                                emit_filler()

                def norm(s, av, q0, W):
                    """Divide av[0:64] by the ones-row sum; write hoT."""
                    po = (s % 2) * 64
                    hoT_s = hoT[s // 2]
                    lr = nrm.tile([65, W], F32R, tag="lr", name="lr")
                    nc.vector.tensor_copy(lr[64:65, :], av[64:65, :])
                    bps = bp_ps.tile([64, W], F32, tag="bps", name="bps")
                    nc.tensor.matmul(
                        bps[:], ones_r[64:65, 0:64].bitcast(F32R),
                        lr[64:65, :])
                    binv = nrm.tile([64, W], F32, tag="binv", name="binv")
                    nc.vector.reciprocal_approx_fast(out=binv[:], in_=bps[:])
                    if po == 0:
                        nc.vector.tensor_mul(
                            hoT_s[0:64, q0:q0 + W], av[0:64, :], binv[:])
                    else:
                        # DVE lanes can't shift partitions; bounce via DMA
                        tmp = nrm.tile([64, W], BF16, tag="hotmp",
                                       name="hotmp")
                        nc.vector.tensor_mul(tmp[:], av[0:64, :], binv[:])
                        # SP queue: keeps the Pool queue free for the
                        # diagonal-mask affine_selects
                        nc.sync.dma_start(
                            hoT_s[64:128, q0:q0 + W], tmp[:])

                def quad_d(g, avD):
                    for i4 in range(4):
                        yield from scores_av(3, sched[3][4 * g + i4], avD,
                                             i4 * 128)

                def op_block(sb):
                    ob = obp.tile([128, D], BF16, tag="ob", name="ob")
                    # cc-outer: consecutive matmuls share the stationary
                    # hoT slice (halves the LDWEIGHTS traffic on HW).
                    # Tail blocks alternate between the freed sc pool and
                    # the op pool so drains of consecutive blocks overlap.
                    pool = sc_ps if (sb >= 12 and sb % 2 == 0) else op_ps
                    tag = "sc" if pool is sc_ps else "op"
                    pss = [pool.tile([128, 512], F32, tag=tag, name="op")
                           for _ in range(2)]
                    for cc in range(2):
                        for jh in range(2):
                            nc.tensor.matmul(
                                pss[jh][:],
                                hoT[cc][:, sb * 128:(sb + 1) * 128],
                                wo_sb[:, cc * D + jh * 512:
                                      cc * D + (jh + 1) * 512],
                                start=(cc == 0), stop=(cc == 1),
                            )
                    if sb >= 12:
                        # tail: drain halves on ACT/DVE and ship each half
                        # immediately on its own DMA queue
                        nc.scalar.copy(ob[:, 0:512], pss[0][:])
                        nc.sync.dma_start(out[sb * 128:(sb + 1) * 128, 0:512],
                                          ob[:, 0:512])
                        nc.vector.tensor_copy(ob[:, 512:1024], pss[1][:])
                        nc.gpsimd.dma_start(
                            out[sb * 128:(sb + 1) * 128, 512:1024],
                            ob[:, 512:1024])
                    else:
                        for jh in range(2):
                            nc.vector.tensor_copy(
                                ob[:, jh * 512:(jh + 1) * 512], pss[jh][:])
                        nc.sync.dma_start(out[sb * 128:(sb + 1) * 128, :],
                                          ob[:])

                for g in range(4):
                    nfill = 1 if g < 3 else 2
                    # slot D first: its norm bounces hoT rows through a
                    # DMA whose latency then hides under slots A-C
                    avD = av_ps.tile([65, 512], F32, tag="av", name="avD")
                    for i4 in range(4):
                        run_gens([scores_av(3, sched[3][4 * g + i4], avD,
                                            i4 * 128)], 10**6)
                        for _ in range(nfill):
                            emit_filler()
                    norm(3, avD, g * 512, 512)
                    for s in (0, 1, 2):
                        avS = av_ps.tile([65, 512], F32, tag="av",
                                         name="avS")
                        run_gens([scores_av(s, sched[s][g], avS, 0)], 10**6)
                        norm(s, avS, g * 512, 512)
                        for _ in range(nfill):
                            emit_filler()
                    for sb in range(4 * g, 4 * g + 4):
                        fillers.append(("op", sb))
                while fillers:
                    emit_filler()

    nc.compile()
    return nc


def make_core_inputs(c, x, W_packed, b_packed, W_out):
    """Host-side shard prep for core c (pure numpy reshuffles)."""
    import ml_dtypes
    k, b = c % 4, c // 4
    heads = [12 + k, 8 + k, 4 + k, k]          # slots A..D
    rows = np.concatenate([np.arange(h * 64, (h + 1) * 64) for h in heads])

    xTc = np.ascontiguousarray(x[b].T)                      # [D, S]
    wq = W_packed[rows]                                     # [256, D]
    wk = W_packed[D + rows]
    wv = W_packed[2 * D + rows]
    wqkvT = np.ascontiguousarray(
        np.concatenate([wq, wk, wv], 0).T)                  # [D, 768]

    woTc = np.ascontiguousarray(W_out[:, rows].T)           # [256, D]
    woP = np.concatenate([woTc[:128], woTc[128:]], axis=1)  # [128, 2D]

    bq = b_packed[rows] / 8.0
    bk = b_packed[D + rows]
    bqk = np.stack([bq[:128], bq[128:], bk[:128], bk[128:]], 1)  # [128, 4]

    btabq = np.zeros((128, TABW + 4), np.float32)
    p = np.arange(128, dtype=np.float64)[:, None]
    for s in range(4):
        h = heads[s]
        slope = 2.0 ** (-(h + 1) * 8.0 / H)
        K, off0, tw, to = (SLOT_KEEP[s], SLOT_OFF0[s], SLOT_TABW[s],
                           SLOT_TABOFF[s])
        m = np.arange(tw, dtype=np.float64)[None, :]
        btabq[:, to:to + tw] = (slope * (p + 128.0 * (m - (K - 1)) - off0)
                                ).astype(np.float32)
    btabq[:, TABW:] = bqk.astype(np.float32)
    return heads, {"xT": xTc.astype(ml_dtypes.bfloat16),
                   "wqkvT": wqkvT.astype(ml_dtypes.bfloat16),
                   "woT": woP.astype(ml_dtypes.bfloat16),
                   "btabq": btabq}


_NC_CACHE = {}


def _get_program():
    if "nc" not in _NC_CACHE:
        _NC_CACHE["nc"] = build_program()
    return _NC_CACHE["nc"]


def make_in_maps(x, W_packed, b_packed, W_out):
    return [make_core_inputs(c, x, W_packed, b_packed, W_out)[1]
            for c in range(NCORES)]


def kernel(x, W_packed, b_packed, W_out, b_out):
    x = np.asarray(x, np.float32)
    W_packed = np.asarray(W_packed, np.float32)
    b_packed = np.asarray(b_packed, np.float32)
    W_out = np.asarray(W_out, np.float32)
    b_out = np.asarray(b_out, np.float32)

    nc = _get_program()
    in_maps = make_in_maps(x, W_packed, b_packed, W_out)
    res = run_bass_kernel_spmd(nc, in_maps, core_ids=list(range(NCORES)))

    # Gather: sum partials per batch; add b_out and the folded v-bias term.
    b_v = b_packed[2 * D:]
    bias_row = (b_out + W_out @ b_v).astype(np.float32)     # [D]
    full = np.empty((B, S, D), np.float32)
    for b in range(B):
        acc = res.results[4 * b]["out"].astype(np.float32).copy()
        for c in range(4 * b + 1, 4 * b + 4):
            acc += res.results[c]["out"]
        full[b] = acc + bias_row
    return full


# revision 63
# speedup vs baseline: 1.0846x; 1.0057x over previous
"""Trainium2 Bass kernel for causal multi-head attention with ALiBi.

Computes, for x:[B,S,D]:
    qkv = x @ W_packed.T + b_packed ; q,k,v = split(qkv)
    heads -> scores = q k^T / sqrt(hd) + alibi_causal_bias
    out = softmax(scores) v -> merge heads -> out @ W_out.T + b_out

Sharding (8 cores): core c handles batch c//4 and heads {k, k+4, k+8, k+12}
(k = c%4), one head per "slot". Slot block-schedules are head-independent
(sized for the largest ALiBi window in the slot), so one SPMD program runs
on all 8 cores; only the data (weight slices, bias tables) differs.
Host sums the 4 out-projection partials per batch and adds
b_out + W_out @ b_v (the v-bias term commutes through attention).

ALiBi sparsity: head h attends effectively only a bounded window back;
dropped softmax mass is <= e^-8 at the worst (group-start) columns.
Slots keep only the causal k-blocks within that window (KEEP blocks).

Softmax without row-max: scores are O(+-6), and exp is recentred per
q-group by C_g (a per-column constant that cancels in normalization),
keeping exp args in fp32 range. In the transposed layout scoresT[k,q] the
recentred ALiBi bias slope*(k - C_g) is per-partition, so it rides the
single Exp activation for free. Row sums come from a ones-row appended to
v in the attn@v matmul; normalization divides by that row.
"""

import os
import sys

import numpy as np

for _p in ("/opt/trn_rl_repo",):
    if os.path.isdir(_p) and _p not in sys.path:
        sys.path.append(_p)

import concourse.bacc as bacc
import concourse.bass as bass
import concourse.tile as tile
from concourse import mybir
from concourse.bass_utils import run_bass_kernel_spmd

B, S, D, H, HD = 2, 2048, 1024, 16, 64
NBLK = S // 128          # 16 k/q blocks
NCORES = 8

F32 = mybir.dt.float32
F32R = mybir.dt.float32r
BF16 = mybir.dt.bfloat16

# Slots A..D: per-core heads [12+k, 8+k, 4+k, k].  KEEP = causal k-blocks
# kept per q-block (max over the slot's heads).  W = q-group width.
SLOT_KEEP = (17, 5, 5, 2)
SLOT_W = (512, 512, 512, 128)
SLOT_OFF0 = (128, 128, 128, 64)
SLOT_TABW = tuple(k + 3 if w == 512 else k for k, w in zip(SLOT_KEEP, SLOT_W))
SLOT_TABOFF = tuple(int(np.cumsum((0,) + SLOT_TABW)[i]) for i in range(4))
TABW = int(sum(SLOT_TABW))


def _slot_schedule(s):
    """Yield (g, q0, W, [(j, lo, hi, tabcol, isdiag), ...]) per q-group."""
    K, W, _ = SLOT_KEEP[s], SLOT_W[s], SLOT_OFF0[s]
    out = []
    if W == 512:
        for g in range(S // 512):
            jlo = max(0, 4 * g + 3 - (K - 1))
            blocks = []
            for j in range(jlo, 4 * g + 4):
                lo = max(0, (j - 4 * g) * 128)
                hi = min(512, (j - 4 * g + K) * 128)
                m = j - 4 * g + (K - 1)
                blocks.append((j, lo, hi, SLOT_TABOFF[s] + m, j >= 4 * g))
            out.append((g, g * 512, 512, blocks))
    else:
        for i in range(NBLK):
            blocks = []
            for j in range(max(0, i - (K - 1)), i + 1):
                m = j - i + (K - 1)
                blocks.append((j, 0, 128, SLOT_TABOFF[s] + m, j == i))
            out.append((i, i * 128, 128, blocks))
    return out


def build_program():
    nc = bacc.Bacc("TRN2", target_bir_lowering=False, debug=False,
                   num_devices=NCORES)

    xT = nc.dram_tensor("xT", [D, S], BF16, kind="ExternalInput")
    wqkvT = nc.dram_tensor("wqkvT", [D, 768], BF16, kind="ExternalInput")
    woT = nc.dram_tensor("woT", [128, 2 * D], BF16, kind="ExternalInput")
    btabq = nc.dram_tensor("btabq", [128, TABW + 4], F32, kind="ExternalInput")
    out = nc.dram_tensor("out", [S, D], BF16, kind="ExternalOutput")

    with tile.TileContext(nc) as tc:
        with tc.tile_pool(name="persist", bufs=1) as pp:
            qkT = [pp.tile([128, S], BF16, tag=f"qkT{t}", name=f"qkT{t}")
                   for t in range(4)]
            v_t = pp.tile([128, 4, NBLK, 65], BF16, tag="v", name="v")
            hoT = [pp.tile([128, S], BF16, tag=f"hoT{t}", name=f"hoT{t}")
                   for t in range(2)]
            btab_sb = pp.tile([128, TABW + 4], F32, tag="btab", name="btab")
            ones_r = pp.tile([65, 64], F32, tag="ones_r", name="ones_r")
            wo_sb = pp.tile([128, 2 * D], BF16, tag="wo", name="wo")

            nc.gpsimd.memset(v_t[:, :, :, 64:65], 1.0)
            nc.gpsimd.memset(ones_r[:], 1.0)
            # touch the ACT engine once while it is idle so the activation
            # table load happens here, not on the first real Exp/Identity
            warm = pp.tile([1, 1], F32, tag="warm", name="warm")
            nc.scalar.activation(warm[:], ones_r[0:1, 0:1],
                                 mybir.ActivationFunctionType.Exp)

            # PSUM: 8 banks as 4 tags; phase-1 QKV borrows all four tags
            with (
                tc.tile_pool(name="xw", bufs=1) as xw,
                tc.tile_pool(name="et", bufs=8) as etp,
                tc.tile_pool(name="nrm", bufs=3) as nrm,
                tc.tile_pool(name="ob", bufs=2) as obp,
                tc.tile_pool(name="ps_sc", bufs=3, space="PSUM") as sc_ps,
                tc.tile_pool(name="ps_av", bufs=2, space="PSUM") as av_ps,
                tc.tile_pool(name="ps_bp", bufs=1, space="PSUM") as bp_ps,
                tc.tile_pool(name="ps_op", bufs=2, space="PSUM") as op_ps,
            ):
                # input stream: (wqkv_m, x_m) pairs first -- qkv phase 1
                # consumes chunk-major right behind the stream -- then the
                # small tables and the out-proj weight (needed much later).
                # weights ride the Pool DMA queue, x the SP queue: the two
                # streams overlap (aggregate stays under the HBM cap since
                # the weight stream is a third of the x stream)
                xT_sb, wqkv_sb = [], []
                for m in range(8):
                    t = xw.tile([128, 768], BF16, tag=f"wqkv{m}",
                                name=f"wqkv{m}")
                    nc.gpsimd.dma_start(t[:], wqkvT[m * 128:(m + 1) * 128, :])
                    wqkv_sb.append(t)
                    t = xw.tile([128, S], BF16, tag=f"x{m}", name=f"x{m}")
                    nc.sync.dma_start(t[:], xT[m * 128:(m + 1) * 128, :])
                    xT_sb.append(t)
                nc.gpsimd.dma_start(btab_sb[:], btabq[:])
                nc.gpsimd.dma_start(wo_sb[:], woT[:])

                def p1_tile(i, w):
                    pool, tag = [(sc_ps, "sc"), (sc_ps, "sc"), (sc_ps, "sc"),
                                 (bp_ps, "bps"), (av_ps, "av"), (av_ps, "av"),
                                 (op_ps, "op"), (op_ps, "op")][i]
                    return pool.tile([128, w], F32, tag=tag, name=f"p1_{i}")

                def qk_half(half):
                    # quarters {2h,2h+1} x 4 f-tiles -> 8 one-bank psums
                    # (m-outer: first matmul waits only for chunk-0 DMAs)
                    pss = {}
                    for ft in range(4):
                        for qi in range(2):
                            pss[ft, qi] = p1_tile(ft * 2 + qi, 512)
                    for m in range(8):
                        for ft in range(4):
                            for qi in range(2):
                                q4 = half * 2 + qi
                                nc.tensor.matmul(
                                    pss[ft, qi][:],
                                    wqkv_sb[m][:, ft * 128:(ft + 1) * 128],
                                    xT_sb[m][:, q4 * 512:(q4 + 1) * 512],
                                    start=(m == 0), stop=(m == 7),
                                )
                    # drain ft=3 first: its psums sit on the "op" tag the
                    # v/qk1 filler units need next.  ft=3 and qi=1 drains
                    # ride ACT (idle here) to shorten the DVE chain.
                    for ft in (3, 0, 1, 2):
                        for qi in range(2):
                            q4 = half * 2 + qi
                            scol = slice(q4 * 512, (q4 + 1) * 512)
                            # psum*scale + bias (1/sqrt(hd) folded into q)
                            if qi == 1:
                                nc.scalar.activation(
                                    qkT[ft][:, scol], pss[ft, qi][:],
                                    mybir.ActivationFunctionType.Identity,
                                    bias=btab_sb[:, TABW + ft:TABW + ft + 1],
                                    scale=(0.125 if ft < 2 else 1.0),
                                )
                            else:
                                nc.vector.tensor_scalar(
                                    out=qkT[ft][:, scol], in0=pss[ft, qi][:],
                                    scalar1=(0.125 if ft < 2 else 1.0),
                                    scalar2=btab_sb[:, TABW + ft:
                                                    TABW + ft + 1],
                                    op0=mybir.AluOpType.mult,
                                    op1=mybir.AluOpType.add,
                                )

                def v_unit(sb):
                    # one k-block of v for all 4 slots; m-inner, 1/2 bank
                    ps = op_ps.tile([128, 256], F32, tag="op", name="vps")
                    for m in range(8):
                        nc.tensor.matmul(
                            ps[:],
                            xT_sb[m][:, sb * 128:(sb + 1) * 128],
                            wqkv_sb[m][:, 512:768],
                            start=(m == 0), stop=(m == 7),
                        )
                    nc.vector.tensor_copy(
                        v_t[:, :, sb, 0:64],
                        ps[:].rearrange("p (s c) -> p s c", s=4),
                    )

                def qk1_unit(ft, qi):
                    # one (f-tile, q-quarter) of the second qk half; 1 bank
                    q4 = 2 + qi
                    ps = op_ps.tile([128, 512], F32, tag="op", name="qk1ps")
                    for m in range(8):
                        nc.tensor.matmul(
                            ps[:],
                            wqkv_sb[m][:, ft * 128:(ft + 1) * 128],
                            xT_sb[m][:, q4 * 512:(q4 + 1) * 512],
                            start=(m == 0), stop=(m == 7),
                        )
                    scol = slice(q4 * 512, (q4 + 1) * 512)
                    if ft >= 2:
                        nc.scalar.activation(
                            qkT[ft][:, scol], ps[:],
                            mybir.ActivationFunctionType.Identity,
                            bias=btab_sb[:, TABW + ft:TABW + ft + 1],
                            scale=1.0,
                        )
                    else:
                        nc.vector.tensor_scalar(
                            out=qkT[ft][:, scol], in0=ps[:],
                            scalar1=0.125,
                            scalar2=btab_sb[:, TABW + ft:TABW + ft + 1],
                            op0=mybir.AluOpType.mult,
                            op1=mybir.AluOpType.add,
                        )

                qk_half(0)
                for sb in range(4):
                    v_unit(sb)

                # filler units: PE work spliced into the exp-paced attention
                # stream.  Order respects data deps (g2 needs qk1 qi=0 and
                # v 8-11, g3 needs qi=1 and v 12-15; all are emitted at
                # least one full q-group before first use).
                from collections import deque
                fillers = deque()
                for sb in range(4, 8):
                    fillers.append(("v", sb))
                for ft in range(4):
                    fillers.append(("qk1", ft, 0))
                for sb in range(8, 12):
                    fillers.append(("v", sb))
                for ft in range(4):
                    fillers.append(("qk1", ft, 1))
                for sb in range(12, 16):
                    fillers.append(("v", sb))

                def emit_filler():
                    if not fillers:
                        return
                    u = fillers.popleft()
                    if u[0] == "v":
                        v_unit(u[1])
                    elif u[0] == "qk1":
                        qk1_unit(u[1], u[2])
                    else:
                        op_block(u[1])

                # ---- attention (+ out-proj interleaved per q-group) ----
                sched = [_slot_schedule(s) for s in range(4)]

                def scores_av(s, ent, av, coff):
                    """Scores+exp+AV for one q-group into av[:, coff:+W].

                    Generator, two yields per block: after the score matmul
                    and after the (previous block's) AV matmul.  Pairing two
                    slots' generators puts their 64-contraction score
                    matmuls back-to-back on disjoint PE row groups (slot
                    parity gives base partitions 0/64), so they execute
                    concurrently on HW.  The av-before-sc offset is also a
                    depth-1 software pipeline past the exp dependency."""
                    po = (s % 2) * 64
                    qT_s = qkT[s // 2][po:po + 64, :]
                    kT_s = qkT[2 + s // 2][po:po + 64, :]
                    g, q0, W, blocks = ent
                    pend = None
                    for bi, (j, lo, hi, tcol, isdiag) in enumerate(blocks):
                        # partial widths: block 0 always spans [0:W] (sets
                        # has_written on the full av range); later blocks
                        # touch only their causal window [lo:hi].
                        sc = sc_ps.tile([128, W], F32, tag="sc", name="sc")
                        nc.tensor.matmul(
                            sc[:, lo:hi],
                            kT_s[:, j * 128:(j + 1) * 128],
                            qT_s[:, q0 + lo:q0 + hi],
                        )
                        et = etp.tile([128, W], BF16, tag="et", name="et")
                        nc.scalar.activation(
                            et[:, lo:hi], sc[:, lo:hi],
                            mybir.ActivationFunctionType.Exp,
                            bias=btab_sb[:, tcol:tcol + 1], scale=1.0,
                        )
                        if isdiag:
                            # zero k>q inside the diagonal 128x128 block
                            nc.gpsimd.affine_select(
                                out=et[:, lo:lo + 128],
                                in_=et[:, lo:lo + 128],
                                compare_op=mybir.AluOpType.is_ge,
                                fill=0.0, base=0,
                                pattern=[[1, 128]],
                                channel_multiplier=-1,
                            )
                        yield
                        if pend is not None:
                            pbi, pj, plo, phi, pet = pend
                            nc.tensor.matmul(
                                av[:, coff + plo:coff + phi],
                                v_t[:, s, pj, :], pet[:, plo:phi],
                                start=(pbi == 0), stop=False,
                            )
                        pend = (bi, j, lo, hi, et)
                        yield
                    pbi, pj, plo, phi, pet = pend
                    nc.tensor.matmul(
                        av[:, coff + plo:coff + phi], v_t[:, s, pj, :],
                        pet[:, plo:phi], start=(pbi == 0), stop=True,
                    )

                def run_gens(gens, fill_every=6):
                    """Round-robin the slot generators; splice a filler
                    unit into the PE stream every `fill_every` steps."""
                    gens = list(gens)
                    steps = 0
                    while gens:
                        for gx in list(gens):
                            try:
                                next(gx)
                            except StopIteration:
                                gens.remove(gx)
                                continue
                            steps += 1
                            if steps % fill_every == 0:
                                emit_filler()

                def norm(s, av, q0, W):
                    """Divide av[0:64] by the ones-row sum; write hoT."""
                    po = (s % 2) * 64
                    hoT_s = hoT[s // 2]
                    lr = nrm.tile([65, W], F32R, tag="lr", name="lr")
                    nc.vector.tensor_copy(lr[64:65, :], av[64:65, :])
                    bps = bp_ps.tile([64, W], F32, tag="bps", name="bps")
                    nc.tensor.matmul(
                        bps[:], ones_r[64:65, 0:64].bitcast(F32R),
                        lr[64:65, :])
                    binv = nrm.tile([64, W], F32, tag="binv", name="binv")
                    nc.vector.reciprocal_approx_fast(out=binv[:], in_=bps[:])
                    if po == 0:
                        nc.vector.tensor_mul(
                            hoT_s[0:64, q0:q0 + W], av[0:64, :], binv[:])
                    else:
                        # DVE lanes can't shift partitions; bounce via DMA
                        tmp = nrm.tile([64, W], BF16, tag="hotmp",
                                       name="hotmp")
                        nc.vector.tensor_mul(tmp[:], av[0:64, :], binv[:])
                        # SP queue: keeps the Pool queue free for the
                        # diagonal-mask affine_selects
                        nc.sync.dma_start(
                            hoT_s[64:128, q0:q0 + W], tmp[:])

                def quad_d(g, avD):
                    for i4 in range(4):
                        yield from scores_av(3, sched[3][4 * g + i4], avD,
                                             i4 * 128)

                def op_block(sb):
                    ob = obp.tile([128, D], BF16, tag="ob", name="ob")
                    # cc-outer: consecutive matmuls share the stationary
                    # hoT slice (halves the LDWEIGHTS traffic on HW).
                    # Tail blocks alternate between the freed sc pool and
                    # the op pool so drains of consecutive blocks overlap.
                    pool = sc_ps if (sb >= 12 and sb % 2 == 0) else op_ps
                    tag = "sc" if pool is sc_ps else "op"
                    pss = [pool.tile([128, 512], F32, tag=tag, name="op")
                           for _ in range(2)]
                    for cc in range(2):
                        for jh in range(2):
                            nc.tensor.matmul(
                                pss[jh][:],
                                hoT[cc][:, sb * 128:(sb + 1) * 128],
                                wo_sb[:, cc * D + jh * 512:
                                      cc * D + (jh + 1) * 512],
                                start=(cc == 0), stop=(cc == 1),
                            )
                    if sb >= 12:
                        # tail: drain halves on ACT/DVE and ship each half
                        # immediately on its own DMA queue
                        nc.scalar.copy(ob[:, 0:512], pss[0][:])
                        nc.sync.dma_start(out[sb * 128:(sb + 1) * 128, 0:512],
                                          ob[:, 0:512])
                        nc.vector.tensor_copy(ob[:, 512:1024], pss[1][:])
                        nc.gpsimd.dma_start(
                            out[sb * 128:(sb + 1) * 128, 512:1024],
                            ob[:, 512:1024])
                    else:
                        for jh in range(2):
                            nc.vector.tensor_copy(
                                ob[:, jh * 512:(jh + 1) * 512], pss[jh][:])
                        nc.sync.dma_start(out[sb * 128:(sb + 1) * 128, :],
                                          ob[:])

                for g in range(4):
                    nfill = 1 if g < 3 else 2
                    # slot D first: its norm bounces hoT rows through a
                    # DMA whose latency then hides under slots A-C
                    avD = av_ps.tile([65, 512], F32, tag="av", name="avD")
                    for i4 in range(4):
                        run_gens([scores_av(3, sched[3][4 * g + i4], avD,
                                            i4 * 128)], 10**6)
                        for _ in range(nfill):
                            emit_filler()
                    norm(3, avD, g * 512, 512)
                    for s in (0, 1, 2):
                        avS = av_ps.tile([65, 512], F32, tag="av",
                                         name="avS")
                        run_gens([scores_av(s, sched[s][g], avS, 0)], 10**6)
                        norm(s, avS, g * 512, 512)
                        for _ in range(nfill):
                            emit_filler()
                    for sb in range(4 * g, 4 * g + 4):
                        fillers.append(("op", sb))
                while fillers:
                    emit_filler()

    nc.compile()
    return nc


def make_core_inputs(c, x, W_packed, b_packed, W_out):
    """Host-side shard prep for core c (pure numpy reshuffles)."""
    import ml_dtypes
    k, b = c % 4, c // 4
    heads = [12 + k, 8 + k, 4 + k, k]          # slots A..D
    rows = np.concatenate([np.arange(h * 64, (h + 1) * 64) for h in heads])

    xTc = np.ascontiguousarray(x[b].T)                      # [D, S]
    wq = W_packed[rows]                                     # [256, D]
    wk = W_packed[D + rows]
    wv = W_packed[2 * D + rows]
    wqkvT = np.ascontiguousarray(
        np.concatenate([wq, wk, wv], 0).T)                  # [D, 768]

    woTc = np.ascontiguousarray(W_out[:, rows].T)           # [256, D]
    woP = np.concatenate([woTc[:128], woTc[128:]], axis=1)  # [128, 2D]

    bq = b_packed[rows] / 8.0
    bk = b_packed[D + rows]
    bqk = np.stack([bq[:128], bq[128:], bk[:128], bk[128:]], 1)  # [128, 4]

    btabq = np.zeros((128, TABW + 4), np.float32)
    p = np.arange(128, dtype=np.float64)[:, None]
    for s in range(4):
        h = heads[s]
        slope = 2.0 ** (-(h + 1) * 8.0 / H)
        K, off0, tw, to = (SLOT_KEEP[s], SLOT_OFF0[s], SLOT_TABW[s],
                           SLOT_TABOFF[s])
        m = np.arange(tw, dtype=np.float64)[None, :]
        btabq[:, to:to + tw] = (slope * (p + 128.0 * (m - (K - 1)) - off0)
                                ).astype(np.float32)
    btabq[:, TABW:] = bqk.astype(np.float32)
    return heads, {"xT": xTc.astype(ml_dtypes.bfloat16),
                   "wqkvT": wqkvT.astype(ml_dtypes.bfloat16),
                   "woT": woP.astype(ml_dtypes.bfloat16),
                   "btabq": btabq}


_NC_CACHE = {}


def _get_program():
    if "nc" not in _NC_CACHE:
        _NC_CACHE["nc"] = build_program()
    return _NC_CACHE["nc"]


def make_in_maps(x, W_packed, b_packed, W_out):
    return [make_core_inputs(c, x, W_packed, b_packed, W_out)[1]
            for c in range(NCORES)]


def kernel(x, W_packed, b_packed, W_out, b_out):
    x = np.asarray(x, np.float32)
    W_packed = np.asarray(W_packed, np.float32)
    b_packed = np.asarray(b_packed, np.float32)
    W_out = np.asarray(W_out, np.float32)
    b_out = np.asarray(b_out, np.float32)

    nc = _get_program()
    in_maps = make_in_maps(x, W_packed, b_packed, W_out)
    res = run_bass_kernel_spmd(nc, in_maps, core_ids=list(range(NCORES)))

    # Gather: sum partials per batch; add b_out and the folded v-bias term.
    b_v = b_packed[2 * D:]
    bias_row = (b_out + W_out @ b_v).astype(np.float32)     # [D]
    full = np.empty((B, S, D), np.float32)
    for b in range(B):
        acc = res.results[4 * b]["out"].astype(np.float32).copy()
        for c in range(4 * b + 1, 4 * b + 4):
            acc += res.results[c]["out"]
        full[b] = acc + bias_row
    return full


# revision 70
# speedup vs baseline: 1.0995x; 1.0137x over previous
"""Trainium2 Bass kernel for causal multi-head attention with ALiBi.

Computes, for x:[B,S,D]:
    qkv = x @ W_packed.T + b_packed ; q,k,v = split(qkv)
    heads -> scores = q k^T / sqrt(hd) + alibi_causal_bias
    out = softmax(scores) v -> merge heads -> out @ W_out.T + b_out

Sharding (8 cores): core c handles batch c//4 and heads {k, k+4, k+8, k+12}
(k = c%4), one head per "slot". Slot block-schedules are head-independent
(sized for the largest ALiBi window in the slot), so one SPMD program runs
on all 8 cores; only the data (weight slices, bias tables) differs.
Host sums the 4 out-projection partials per batch and adds
b_out + W_out @ b_v (the v-bias term commutes through attention).

ALiBi sparsity: head h attends effectively only a bounded window back;
dropped softmax mass is <= e^-8 at the worst (group-start) columns.
Slots keep only the causal k-blocks within that window (KEEP blocks).

Softmax without row-max: scores are O(+-6), and exp is recentred per
q-group by C_g (a per-column constant that cancels in normalization),
keeping exp args in fp32 range. In the transposed layout scoresT[k,q] the
recentred ALiBi bias slope*(k - C_g) is per-partition, so it rides the
single Exp activation for free. Row sums come from a ones-row appended to
v in the attn@v matmul; normalization divides by that row.
"""

import os
import sys

import numpy as np

for _p in ("/opt/trn_rl_repo",):
    if os.path.isdir(_p) and _p not in sys.path:
        sys.path.append(_p)

import concourse.bacc as bacc
import concourse.bass as bass
import concourse.tile as tile
from concourse import mybir
from concourse.bass_utils import run_bass_kernel_spmd

B, S, D, H, HD = 2, 2048, 1024, 16, 64
NBLK = S // 128          # 16 k/q blocks
NCORES = 8

F32 = mybir.dt.float32
F32R = mybir.dt.float32r
BF16 = mybir.dt.bfloat16

# Slots A..D: per-core heads [12+k, 8+k, 4+k, k].  KEEP = causal k-blocks
# kept per q-block (max over the slot's heads).  W = q-group width.
SLOT_KEEP = (17, 6, 5, 2)
SLOT_W = (512, 512, 512, 128)
SLOT_OFF0 = (128, 128, 128, 64)
SLOT_TABW = tuple(k + 3 if w == 512 else k for k, w in zip(SLOT_KEEP, SLOT_W))
SLOT_TABOFF = tuple(int(np.cumsum((0,) + SLOT_TABW)[i]) for i in range(4))
TABW = int(sum(SLOT_TABW))


def _slot_schedule(s):
    """Yield (g, q0, W, [(j, lo, hi, tabcol, isdiag), ...]) per q-group."""
    K, W, _ = SLOT_KEEP[s], SLOT_W[s], SLOT_OFF0[s]
    out = []
    if W == 512:
        for g in range(S // 512):
            jlo = max(0, 4 * g + 3 - (K - 1))
            blocks = []
            for j in range(jlo, 4 * g + 4):
                lo = max(0, (j - 4 * g) * 128)
                hi = min(512, (j - 4 * g + K) * 128)
                m = j - 4 * g + (K - 1)
                blocks.append((j, lo, hi, SLOT_TABOFF[s] + m, j >= 4 * g))
            out.append((g, g * 512, 512, blocks))
    else:
        for i in range(NBLK):
            blocks = []
            for j in range(max(0, i - (K - 1)), i + 1):
                m = j - i + (K - 1)
                blocks.append((j, 0, 128, SLOT_TABOFF[s] + m, j == i))
            out.append((i, i * 128, 128, blocks))
    return out


def build_program():
    nc = bacc.Bacc("TRN2", target_bir_lowering=False, debug=False,
                   num_devices=NCORES)

    xT = nc.dram_tensor("xT", [D, S], BF16, kind="ExternalInput")
    wqkvT = nc.dram_tensor("wqkvT", [D, 768], BF16, kind="ExternalInput")
    woT = nc.dram_tensor("woT", [128, 2 * D], BF16, kind="ExternalInput")
    btabq = nc.dram_tensor("btabq", [128, TABW + 4], F32, kind="ExternalInput")
    out = nc.dram_tensor("out", [S, D], BF16, kind="ExternalOutput")

    with tile.TileContext(nc) as tc:
        with tc.tile_pool(name="persist", bufs=1) as pp:
            qkT = [pp.tile([128, S], BF16, tag=f"qkT{t}", name=f"qkT{t}")
                   for t in range(4)]
            v_t = pp.tile([128, 4, NBLK, 65], BF16, tag="v", name="v")
            hoT = [pp.tile([128, S], BF16, tag=f"hoT{t}", name=f"hoT{t}")
                   for t in range(2)]
            btab_sb = pp.tile([128, TABW + 4], F32, tag="btab", name="btab")
            ones_r = pp.tile([65, 64], F32, tag="ones_r", name="ones_r")
            wo_sb = pp.tile([128, 2 * D], BF16, tag="wo", name="wo")

            nc.gpsimd.memset(v_t[:, :, :, 64:65], 1.0)
            nc.gpsimd.memset(ones_r[:], 1.0)
            # touch the ACT engine once while it is idle so the activation
            # table load happens here, not on the first real Exp/Identity
            warm = pp.tile([1, 1], F32, tag="warm", name="warm")
            nc.scalar.activation(warm[:], ones_r[0:1, 0:1],
                                 mybir.ActivationFunctionType.Exp)

            # PSUM: 8 banks as 4 tags; phase-1 QKV borrows all four tags
            with (
                tc.tile_pool(name="xw", bufs=1) as xw,
                tc.tile_pool(name="et", bufs=8) as etp,
                tc.tile_pool(name="nrm", bufs=3) as nrm,
                tc.tile_pool(name="ob", bufs=4) as obp,
                tc.tile_pool(name="ps_sc", bufs=3, space="PSUM") as sc_ps,
                tc.tile_pool(name="ps_av", bufs=2, space="PSUM") as av_ps,
                tc.tile_pool(name="ps_bp", bufs=1, space="PSUM") as bp_ps,
                tc.tile_pool(name="ps_op", bufs=2, space="PSUM") as op_ps,
            ):
                # input stream: (wqkv_m, x_m) pairs first -- qkv phase 1
                # consumes chunk-major right behind the stream -- then the
                # small tables and the out-proj weight (needed much later).
                # weights ride the Pool DMA queue, x the SP queue: the two
                # streams overlap (aggregate stays under the HBM cap since
                # the weight stream is a third of the x stream)
                xT_sb, wqkv_sb = [], []
                for m in range(8):
                    t = xw.tile([128, 768], BF16, tag=f"wqkv{m}",
                                name=f"wqkv{m}")
                    nc.gpsimd.dma_start(t[:], wqkvT[m * 128:(m + 1) * 128, :])
                    wqkv_sb.append(t)
                    t = xw.tile([128, S], BF16, tag=f"x{m}", name=f"x{m}")
                    # qk half 0 reads only x cols 0-1023; defer the rest
                    nc.sync.dma_start(t[:, 0:1024],
                                      xT[m * 128:(m + 1) * 128, 0:1024])
                    xT_sb.append(t)
                for m in range(8):
                    nc.sync.dma_start(xT_sb[m][:, 1024:2048],
                                      xT[m * 128:(m + 1) * 128, 1024:2048])
                nc.gpsimd.dma_start(btab_sb[:], btabq[:])
                nc.gpsimd.dma_start(wo_sb[:], woT[:])

                def p1_tile(i, w):
                    pool, tag = [(sc_ps, "sc"), (sc_ps, "sc"), (sc_ps, "sc"),
                                 (bp_ps, "bps"), (av_ps, "av"), (av_ps, "av"),
                                 (op_ps, "op"), (op_ps, "op")][i]
                    return pool.tile([128, w], F32, tag=tag, name=f"p1_{i}")

                def qk_half(half):
                    # quarters {2h,2h+1} x 4 f-tiles -> 8 one-bank psums
                    # (m-outer: first matmul waits only for chunk-0 DMAs)
                    pss = {}
                    for ft in range(4):
                        for qi in range(2):
                            pss[ft, qi] = p1_tile(ft * 2 + qi, 512)
                    for m in range(8):
                        for ft in range(4):
                            for qi in range(2):
                                q4 = half * 2 + qi
                                nc.tensor.matmul(
                                    pss[ft, qi][:],
                                    wqkv_sb[m][:, ft * 128:(ft + 1) * 128],
                                    xT_sb[m][:, q4 * 512:(q4 + 1) * 512],
                                    start=(m == 0), stop=(m == 7),
                                )
                    # drain ft=3 first: its psums sit on the "op" tag the
                    # v/qk1 filler units need next.  ft=3 and qi=1 drains
                    # ride ACT (idle here) to shorten the DVE chain.
                    for ft in (3, 0, 1, 2):
                        for qi in range(2):
                            q4 = half * 2 + qi
                            scol = slice(q4 * 512, (q4 + 1) * 512)
                            # psum*scale + bias (1/sqrt(hd) folded into q)
                            if qi == 1:
                                nc.scalar.activation(
                                    qkT[ft][:, scol], pss[ft, qi][:],
                                    mybir.ActivationFunctionType.Identity,
                                    bias=btab_sb[:, TABW + ft:TABW + ft + 1],
                                    scale=(0.125 if ft < 2 else 1.0),
                                )
                            else:
                                nc.vector.tensor_scalar(
                                    out=qkT[ft][:, scol], in0=pss[ft, qi][:],
                                    scalar1=(0.125 if ft < 2 else 1.0),
                                    scalar2=btab_sb[:, TABW + ft:
                                                    TABW + ft + 1],
                                    op0=mybir.AluOpType.mult,
                                    op1=mybir.AluOpType.add,
                                )

                def v_unit(sb):
                    # one k-block of v for all 4 slots; m-inner, 1/2 bank
                    ps = op_ps.tile([128, 256], F32, tag="op", name="vps")
                    for m in range(8):
                        nc.tensor.matmul(
                            ps[:],
                            xT_sb[m][:, sb * 128:(sb + 1) * 128],
                            wqkv_sb[m][:, 512:768],
                            start=(m == 0), stop=(m == 7),
                        )
                    nc.vector.tensor_copy(
                        v_t[:, :, sb, 0:64],
                        ps[:].rearrange("p (s c) -> p s c", s=4),
                    )

                def qk1_unit(ft, qi):
                    # one (f-tile, q-quarter) of the second qk half; 1 bank
                    q4 = 2 + qi
                    ps = op_ps.tile([128, 512], F32, tag="op", name="qk1ps")
                    for m in range(8):
                        nc.tensor.matmul(
                            ps[:],
                            wqkv_sb[m][:, ft * 128:(ft + 1) * 128],
                            xT_sb[m][:, q4 * 512:(q4 + 1) * 512],
                            start=(m == 0), stop=(m == 7),
                        )
                    scol = slice(q4 * 512, (q4 + 1) * 512)
                    if ft >= 2:
                        nc.scalar.activation(
                            qkT[ft][:, scol], ps[:],
                            mybir.ActivationFunctionType.Identity,
                            bias=btab_sb[:, TABW + ft:TABW + ft + 1],
                            scale=1.0,
                        )
                    else:
                        nc.vector.tensor_scalar(
                            out=qkT[ft][:, scol], in0=ps[:],
                            scalar1=0.125,
                            scalar2=btab_sb[:, TABW + ft:TABW + ft + 1],
                            op0=mybir.AluOpType.mult,
                            op1=mybir.AluOpType.add,
                        )

                qk_half(0)
                for sb in range(4):
                    v_unit(sb)

                # filler units: PE work spliced into the exp-paced attention
                # stream.  Order respects data deps (g2 needs qk1 qi=0 and
                # v 8-11, g3 needs qi=1 and v 12-15; all are emitted at
                # least one full q-group before first use).
                from collections import deque
                fillers = deque()
                for sb in range(4, 8):
                    fillers.append(("v", sb))
                for ft in range(4):
                    fillers.append(("qk1", ft, 0))
                for sb in range(8, 12):
                    fillers.append(("v", sb))
                for ft in range(4):
                    fillers.append(("qk1", ft, 1))
                for sb in range(12, 16):
                    fillers.append(("v", sb))

                def emit_filler():
                    if not fillers:
                        return
                    u = fillers.popleft()
                    if u[0] == "v":
                        v_unit(u[1])
                    elif u[0] == "qk1":
                        qk1_unit(u[1], u[2])
                    else:
                        op_block(u[1])

                # ---- attention (+ out-proj interleaved per q-group) ----
                sched = [_slot_schedule(s) for s in range(4)]

                def scores_av(s, ent, av, coff):
                    """Scores+exp+AV for one q-group into av[:, coff:+W].

                    Generator, two yields per block: after the score matmul
                    and after the (previous block's) AV matmul.  Pairing two
                    slots' generators puts their 64-contraction score
                    matmuls back-to-back on disjoint PE row groups (slot
                    parity gives base partitions 0/64), so they execute
                    concurrently on HW.  The av-before-sc offset is also a
                    depth-1 software pipeline past the exp dependency."""
                    po = (s % 2) * 64
                    qT_s = qkT[s // 2][po:po + 64, :]
                    kT_s = qkT[2 + s // 2][po:po + 64, :]
                    g, q0, W, blocks = ent
                    pend = None
                    for bi, (j, lo, hi, tcol, isdiag) in enumerate(blocks):
                        # partial widths: block 0 always spans [0:W] (sets
                        # has_written on the full av range); later blocks
                        # touch only their causal window [lo:hi].
                        sc = sc_ps.tile([128, W], F32, tag="sc", name="sc")
                        nc.tensor.matmul(
                            sc[:, lo:hi],
                            kT_s[:, j * 128:(j + 1) * 128],
                            qT_s[:, q0 + lo:q0 + hi],
                        )
                        et = etp.tile([128, W], BF16, tag="et", name="et")
                        nc.scalar.activation(
                            et[:, lo:hi], sc[:, lo:hi],
                            mybir.ActivationFunctionType.Exp,
                            bias=btab_sb[:, tcol:tcol + 1], scale=1.0,
                        )
                        if isdiag:
                            # zero k>q inside the diagonal 128x128 block
                            nc.gpsimd.affine_select(
                                out=et[:, lo:lo + 128],
                                in_=et[:, lo:lo + 128],
                                compare_op=mybir.AluOpType.is_ge,
                                fill=0.0, base=0,
                                pattern=[[1, 128]],
                                channel_multiplier=-1,
                            )
                        yield
                        if pend is not None:
                            pbi, pj, plo, phi, pet = pend
                            nc.tensor.matmul(
                                av[:, coff + plo:coff + phi],
                                v_t[:, s, pj, :], pet[:, plo:phi],
                                start=(pbi == 0), stop=False,
                            )
                        pend = (bi, j, lo, hi, et)
                        yield
                    pbi, pj, plo, phi, pet = pend
                    nc.tensor.matmul(
                        av[:, coff + plo:coff + phi], v_t[:, s, pj, :],
                        pet[:, plo:phi], start=(pbi == 0), stop=True,
                    )

                def run_gens(gens, fill_every=6):
                    """Round-robin the slot generators; splice a filler
                    unit into the PE stream every `fill_every` steps."""
                    gens = list(gens)
                    steps = 0
                    while gens:
                        for gx in list(gens):
                            try:
                                next(gx)
                            except StopIteration:
                                gens.remove(gx)
                                continue
                            steps += 1
                            if steps % fill_every == 0:
                                emit_filler()

                def norm(s, av, q0, W):
                    """Divide av[0:64] by the ones-row sum; write hoT."""
                    po = (s % 2) * 64
                    hoT_s = hoT[s // 2]
                    lr = nrm.tile([65, W], F32R, tag="lr", name="lr")
                    nc.vector.tensor_copy(lr[64:65, :], av[64:65, :])
                    bps = bp_ps.tile([64, W], F32, tag="bps", name="bps")
                    nc.tensor.matmul(
                        bps[:], ones_r[64:65, 0:64].bitcast(F32R),
                        lr[64:65, :])
                    binv = nrm.tile([64, W], F32, tag="binv", name="binv")
                    nc.vector.reciprocal_approx_fast(out=binv[:], in_=bps[:])
                    if po == 0:
                        nc.vector.tensor_mul(
                            hoT_s[0:64, q0:q0 + W], av[0:64, :], binv[:])
                    else:
                        # DVE lanes can't shift partitions; bounce via DMA
                        tmp = nrm.tile([64, W], BF16, tag="hotmp",
                                       name="hotmp")
                        nc.vector.tensor_mul(tmp[:], av[0:64, :], binv[:])
                        # SP queue: keeps the Pool queue free for the
                        # diagonal-mask affine_selects
                        nc.sync.dma_start(
                            hoT_s[64:128, q0:q0 + W], tmp[:])

                def quad_d(g, avD):
                    for i4 in range(4):
                        yield from scores_av(3, sched[3][4 * g + i4], avD,
                                             i4 * 128)

                def op_block(sb):
                    ob = obp.tile([128, D], BF16, tag="ob", name="ob")
                    # cc-outer: consecutive matmuls share the stationary
                    # hoT slice (halves the LDWEIGHTS traffic on HW).
                    # Tail blocks alternate between the freed sc pool and
                    # the op pool so drains of consecutive blocks overlap.
                    pool = sc_ps if (sb >= 12 and sb % 2 == 0) else op_ps
                    tag = "sc" if pool is sc_ps else "op"
                    pss = [pool.tile([128, 512], F32, tag=tag, name="op")
                           for _ in range(2)]
                    for cc in range(2):
                        for jh in range(2):
                            nc.tensor.matmul(
                                pss[jh][:],
                                hoT[cc][:, sb * 128:(sb + 1) * 128],
                                wo_sb[:, cc * D + jh * 512:
                                      cc * D + (jh + 1) * 512],
                                start=(cc == 0), stop=(cc == 1),
                            )
                    if sb >= 12:
                        # tail: drain halves on ACT/DVE and ship each half
                        # immediately on its own DMA queue
                        nc.scalar.copy(ob[:, 0:512], pss[0][:])
                        nc.sync.dma_start(out[sb * 128:(sb + 1) * 128, 0:512],
                                          ob[:, 0:512])
                        nc.vector.tensor_copy(ob[:, 512:1024], pss[1][:])
                        nc.gpsimd.dma_start(
                            out[sb * 128:(sb + 1) * 128, 512:1024],
                            ob[:, 512:1024])
                    else:
                        for jh in range(2):
                            if jh == 0 and sb >= 8:
                                nc.scalar.copy(ob[:, 0:512], pss[0][:])
                            else:
                                nc.vector.tensor_copy(
                                    ob[:, jh * 512:(jh + 1) * 512],
                                    pss[jh][:])
                        nc.sync.dma_start(out[sb * 128:(sb + 1) * 128, :],
                                          ob[:])

                for g in range(4):
                    nfill = 1 if g < 3 else 2
                    # slot D first: its norm bounces hoT rows through a
                    # DMA whose latency then hides under slots A-C
                    avD = av_ps.tile([65, 512], F32, tag="av", name="avD")
                    for i4 in range(4):
                        run_gens([scores_av(3, sched[3][4 * g + i4], avD,
                                            i4 * 128)], 10**6)
                        for _ in range(nfill):
                            emit_filler()
                    norm(3, avD, g * 512, 512)
                    for s in (0, 1, 2):
                        avS = av_ps.tile([65, 512], F32, tag="av",
                                         name="avS")
                        run_gens([scores_av(s, sched[s][g], avS, 0)], 10**6)
                        norm(s, avS, g * 512, 512)
                        for _ in range(nfill):
                            emit_filler()
                    for sb in range(4 * g, 4 * g + 4):
                        fillers.append(("op", sb))
                while fillers:
                    emit_filler()

    nc.compile()
    return nc


def make_core_inputs(c, x, W_packed, b_packed, W_out):
    """Host-side shard prep for core c (pure numpy reshuffles)."""
    import ml_dtypes
    k, b = c % 4, c // 4
    heads = [12 + k, 8 + k, 4 + k, k]          # slots A..D
    rows = np.concatenate([np.arange(h * 64, (h + 1) * 64) for h in heads])

    xTc = np.ascontiguousarray(x[b].T)                      # [D, S]
    wq = W_packed[rows]                                     # [256, D]
    wk = W_packed[D + rows]
    wv = W_packed[2 * D + rows]
    wqkvT = np.ascontiguousarray(
        np.concatenate([wq, wk, wv], 0).T)                  # [D, 768]

    woTc = np.ascontiguousarray(W_out[:, rows].T)           # [256, D]
    woP = np.concatenate([woTc[:128], woTc[128:]], axis=1)  # [128, 2D]

    bq = b_packed[rows] / 8.0
    bk = b_packed[D + rows]
    bqk = np.stack([bq[:128], bq[128:], bk[:128], bk[128:]], 1)  # [128, 4]

    btabq = np.zeros((128, TABW + 4), np.float32)
    p = np.arange(128, dtype=np.float64)[:, None]
    for s in range(4):
        h = heads[s]
        slope = 2.0 ** (-(h + 1) * 8.0 / H)
        K, off0, tw, to = (SLOT_KEEP[s], SLOT_OFF0[s], SLOT_TABW[s],
                           SLOT_TABOFF[s])
        m = np.arange(tw, dtype=np.float64)[None, :]
        btabq[:, to:to + tw] = (slope * (p + 128.0 * (m - (K - 1)) - off0)
                                ).astype(np.float32)
    btabq[:, TABW:] = bqk.astype(np.float32)
    return heads, {"xT": xTc.astype(ml_dtypes.bfloat16),
                   "wqkvT": wqkvT.astype(ml_dtypes.bfloat16),
                   "woT": woP.astype(ml_dtypes.bfloat16),
                   "btabq": btabq}


_NC_CACHE = {}


def _get_program():
    if "nc" not in _NC_CACHE:
        _NC_CACHE["nc"] = build_program()
    return _NC_CACHE["nc"]


def make_in_maps(x, W_packed, b_packed, W_out):
    return [make_core_inputs(c, x, W_packed, b_packed, W_out)[1]
            for c in range(NCORES)]


def kernel(x, W_packed, b_packed, W_out, b_out):
    x = np.asarray(x, np.float32)
    W_packed = np.asarray(W_packed, np.float32)
    b_packed = np.asarray(b_packed, np.float32)
    W_out = np.asarray(W_out, np.float32)
    b_out = np.asarray(b_out, np.float32)

    nc = _get_program()
    in_maps = make_in_maps(x, W_packed, b_packed, W_out)
    res = run_bass_kernel_spmd(nc, in_maps, core_ids=list(range(NCORES)))

    # Gather: sum partials per batch; add b_out and the folded v-bias term.
    b_v = b_packed[2 * D:]
    bias_row = (b_out + W_out @ b_v).astype(np.float32)     # [D]
    full = np.empty((B, S, D), np.float32)
    for b in range(B):
        acc = res.results[4 * b]["out"].astype(np.float32).copy()
        for c in range(4 * b + 1, 4 * b + 4):
            acc += res.results[c]["out"]
        full[b] = acc + bias_row
    return full
